# revision 16
# baseline (speedup 1.0000x reference)
"""BEVLiftNet Trainium2 kernel (V3: region-batched CCE-add scatter).

Sharding: 8 cores = 2 batches x 4 channel-groups (16 ch each).
Each core lifts all 4 cameras of its batch (depths -> voxel ids), then
scatter-adds 16-channel bf16 feature rows into DRAM grids via indirect
DMA with CCE-add.  Concurrent CCE RMWs to the same address lose
updates (measured), and the per-call gpsimd cost is ~1.7us regardless
of row count (measured), so calls are batched: one call = BF chunks of
128 tokens, chunk j of a call scattering into region j (disjoint
524288-row range) of one grid tensor.  Within-chunk duplicates are
pre-combined with an eq-matrix matmul; redundant/invalid descriptors
are dropped in hardware via the indirect-DMA bounds check (offsets
pushed past the bound by ecnt*DROPK).  Calls alternate between two
region-tensors so two WAW chains pipeline.  Cross-chunk duplicates
land in different regions and are summed by the final merge pass,
which adds all regions, max-reduces over Z, and stores [X*Y,16] f32;
the host reassembles the [1, B*C, X, Y] output.
"""

import os
import sys

sys.path.insert(0, "/opt/trn_rl_repo")

import ml_dtypes
import numpy as np

import concourse.bacc as bacc
import concourse.bass as bass
import concourse.mybir as mybir
import concourse.tile as tile
from concourse import bass_utils
from concourse.masks import make_identity

B, N, C, H, W = 2, 4, 64, 128, 352
X, Y, Z = 256, 256, 8
CG = 16           # channels per core
NCAM = 4          # cameras per core (one batch)
HW = H * W        # 45056 pixels per camera
NTOK = NCAM * HW  # 180224 tokens per core
P = 128
FCAM = HW // P    # 352 chunk-columns per camera
F = NCAM * FCAM   # 1408 chunk-columns total
NROWS = Z * X * Y          # 524288 voxel rows per region
f32 = mybir.dt.float32
bf16 = mybir.dt.bfloat16
i32 = mybir.dt.int32
AL = mybir.AluOpType
ACT = mybir.ActivationFunctionType

BF = int(os.environ.get("KBEV_BF", "1"))        # chunks (regions) per call
NT = int(os.environ.get("KBEV_NT", "4"))        # alternating grid tensors
GD = bf16 if os.environ.get("KBEV_GD", "f32") == "bf16" else f32
GUNROLL = int(os.environ.get("KBEV_UNROLL", "32"))  # groups per loop iter
G = F // BF                                     # scatter groups (calls)
GRID_ROWS = BF * NROWS + BF * P   # + per-(chunk,partition) trash rows
INV = float(BF * NROWS)   # invalid-token id marker (>= all valid ids)
NPDT = ml_dtypes.bfloat16 if GD == bf16 else np.float32
EQD = GD

_prog_cache = None


def _build_program():
    nc = bacc.Bacc("TRN2", target_bir_lowering=False, debug=False)
    feat = nc.dram_tensor("feat", [NTOK, CG], GD, kind="ExternalInput")
    dep = nc.dram_tensor("dep", [NTOK], f32, kind="ExternalInput")
    kmat = nc.dram_tensor("kmat", [NCAM, 3, 3], f32, kind="ExternalInput")
    emat = nc.dram_tensor("emat", [NCAM, 4, 4], f32, kind="ExternalInput")
    tmat_t = nc.dram_tensor("tmat_t", [NCAM, 4, 4], f32, kind="ExternalInput")
    out = nc.dram_tensor("out", [X * Y, CG], f32, kind="ExternalOutput")
    grids = [nc.dram_tensor(f"grid{t}", [GRID_ROWS, CG], GD, kind="Internal")
             for t in range(NT)]
    lin_dram = nc.dram_tensor("lin_dram", [F, P], f32, kind="Internal")
    lin_dram2 = nc.dram_tensor("lin_dram2", [P, F], f32, kind="Internal")
    with tile.TileContext(nc) as tc:
        _emit(tc, feat.ap(), dep.ap(), kmat.ap(), emat.ap(), tmat_t.ap(),
              out.ap(), [g.ap() for g in grids], lin_dram.ap(), lin_dram2.ap())
    nc.compile()
    return nc


def _floor(nc, wp, out_t, in_t, n):
    """out = floor(in), robust to any f32->i32 rounding mode."""
    ii = wp.tile([P, n], i32, tag="fl_i")
    nc.vector.tensor_copy(ii[:], in_t)
    ff = wp.tile([P, n], f32, tag="fl_f")
    nc.vector.tensor_copy(ff[:], ii[:])
    gt = wp.tile([P, n], f32, tag="fl_g")
    nc.vector.tensor_tensor(out=gt[:], in0=ff[:], in1=in_t, op=AL.is_gt)
    nc.vector.tensor_tensor(out=out_t, in0=ff[:], in1=gt[:], op=AL.subtract)


def _emit(tc, feat, dep, kmat, emat, tmat_t, out, grids, lin_dram, lin_dram2):
    nc = tc.nc

    with tc.tile_pool(name="persist", bufs=1) as pp, \
         tc.tile_pool(name="psum", bufs=2, space="PSUM") as psp:

        ident = pp.tile([P, P], f32, tag="ident")
        make_identity(nc, ident[:])
        # lowmask[p, k] = 1.0 where k < p, replicated BF times along free
        iot = pp.tile([P, P], i32, tag="iotpk")
        nc.gpsimd.iota(iot[:], pattern=[[1, P]], base=0, channel_multiplier=-1)
        lowm = pp.tile([P, BF, P], EQD, tag="lowmask")
        for j in range(BF):
            nc.vector.tensor_scalar(out=lowm[:, j, :], in0=iot[:], scalar1=0,
                                    scalar2=None, op0=AL.is_lt)
        ones_col = pp.tile([1, P], f32, tag="ones_col")
        nc.vector.memset(ones_col[:], 1.0)
        # trashc[p, j] = BF*NROWS + j*P + p  (unique in-call trash rows)
        tio = pp.tile([P, BF], i32, tag="tio")
        nc.gpsimd.iota(tio[:], pattern=[[P, BF]], base=BF * NROWS,
                       channel_multiplier=1)
        trashc = pp.tile([P, BF], f32, tag="trashc")
        nc.scalar.copy(trashc[:], tio[:])

        # ---- zero the grid tensors (HWDGE, overlaps with lift) ----
        with tc.tile_pool(name="zpool", bufs=1) as zp:
            ztile = zp.tile([P, 4096], GD, tag="zz")
            nc.vector.memset(ztile[:], 0.0)
            whole = GRID_ROWS * CG
            step = P * 4096
            for g in grids:
                gflat = g.rearrange("a c -> (a c)")
                starts = list(range(0, whole - step + 1, step))
                if starts[-1] + step < whole:
                    starts.append(whole - step)  # overlapped tail, re-zeroed
                for s in starts:
                    nc.sync.dma_start(
                        gflat[s:s + step].rearrange("(p m) -> p m", p=P),
                        ztile[:])

        # ---- pixel-constant tiles (shared by all cameras) ----
        wp_cm = tc.tile_pool(name="work", bufs=6)
        wp = wp_cm.__enter__()
        sp_cm = tc.tile_pool(name="setup", bufs=2)
        sp = sp_cm.__enter__()
        idl = pp.tile([P, FCAM], i32, tag="idl")
        nc.gpsimd.iota(idl[:], pattern=[[1, FCAM]], base=0, channel_multiplier=FCAM)
        idf = pp.tile([P, FCAM], f32, tag="idf")
        nc.scalar.copy(idf[:], idl[:])
        yf = pp.tile([P, FCAM], f32, tag="yf")
        xf = pp.tile([P, FCAM], f32, tag="xf")
        tmp = sp.tile([P, FCAM], f32, tag="t0")
        nc.scalar.activation(tmp[:], idf[:], ACT.Copy, bias=0.5 / W, scale=1.0 / W)
        _floor(nc, sp, yf[:], tmp[:], FCAM)
        nc.vector.scalar_tensor_tensor(out=xf[:], in0=yf[:], scalar=-float(W),
                                       in1=idf[:], op0=AL.mult, op1=AL.add)

        # region offset per chunk-column: regoff[p, f] = (f % BF) * NROWS
        fio = sp.tile([P, F], i32, tag="fio")
        nc.gpsimd.iota(fio[:], pattern=[[1, F]], base=0, channel_multiplier=0)
        fif = sp.tile([P, F], f32, tag="fif")
        nc.scalar.copy(fif[:], fio[:])
        fdiv = sp.tile([P, F], f32, tag="fdiv")
        nc.vector.tensor_scalar(out=fdiv[:], in0=fif[:], scalar1=1.0 / BF,
                                scalar2=None, op0=AL.mult)
        _floor(nc, sp, fdiv[:], fdiv[:], F)
        regoff = pp.tile([P, F], f32, tag="regoff")
        nc.vector.scalar_tensor_tensor(out=regoff[:], in0=fdiv[:],
                                       scalar=-float(BF), in1=fif[:],
                                       op0=AL.mult, op1=AL.add)
        nc.vector.tensor_scalar(out=regoff[:], in0=regoff[:],
                                scalar1=float(NROWS), scalar2=None, op0=AL.mult)

        # ---- per-camera geometry constants ----
        cam_consts = []
        for cam in range(NCAM):
            kc = pp.tile([3, 3], f32, tag=f"kc_{cam}")
            ec = pp.tile([4, 4], f32, tag=f"ec_{cam}")
            tmc = pp.tile([4, 4], f32, tag=f"tc_{cam}")
            nc.sync.dma_start(kc[:], kmat[cam])
            nc.sync.dma_start(ec[:], emat[cam])
            nc.sync.dma_start(tmc[:], tmat_t[cam])
            m4p = psp.tile([4, 4], f32, tag="smallp")
            nc.tensor.matmul(m4p[:], lhsT=tmc[:], rhs=ec[:],
                             start=True, stop=True)
            m4 = pp.tile([4, 4], f32, tag=f"m4_{cam}")
            nc.vector.tensor_copy(m4[:], m4p[:])
            m4tp = psp.tile([4, 4], f32, tag="smallp")
            nc.tensor.transpose(out=m4tp[:], in_=m4[:], identity=ident[:4, :4])
            m4t = pp.tile([4, 4], f32, tag=f"m4t_{cam}")
            nc.vector.tensor_copy(m4t[:], m4tp[:])
            n3p = psp.tile([3, 3], f32, tag="smallp")
            nc.tensor.matmul(n3p[:], lhsT=m4t[:3, :3], rhs=kc[:],
                             start=True, stop=True)
            n3 = pp.tile([3, 3], f32, tag=f"n3_{cam}")
            nc.vector.tensor_copy(n3[:], n3p[:])
            pk = pp.tile([1, 12], f32, tag=f"pk_{cam}")
            for i in range(3):
                nc.gpsimd.dma_start(pk[:, 3 * i:3 * i + 3], n3[i:i + 1, :])
                nc.gpsimd.dma_start(pk[:, 9 + i:10 + i], m4[i:i + 1, 3:4])
            bc = pp.tile([P, 12], f32, tag=f"bc_{cam}")
            nc.gpsimd.partition_broadcast(bc[:], pk[:])
            cam_consts.append(bc)

        # ---- load depths ----
        dtile = pp.tile([P, F], f32, tag="dtile")
        for cam in range(NCAM):
            cs = slice(cam * FCAM, (cam + 1) * FCAM)
            nc.sync.dma_start(
                dtile[:, cs],
                dep[cam * HW:(cam + 1) * HW].rearrange("(p f) -> p f", p=P))

        # ---- lift: voxel linear index per token ----
        linf = pp.tile([P, F], f32, tag="linf")
        nc.vector.memset(linf[:], INV)
        for cam in range(NCAM):
            cs = slice(cam * FCAM, (cam + 1) * FCAM)
            bc = cam_consts[cam]
            d = dtile[:, cs]
            gs = []
            for i in range(3):
                a = sp.tile([P, FCAM], f32, tag="a_i")
                nc.vector.tensor_scalar(out=a[:], in0=xf[:],
                                        scalar1=bc[:, 3 * i:3 * i + 1],
                                        scalar2=None, op0=AL.mult)
                nc.vector.scalar_tensor_tensor(out=a[:], in0=yf[:],
                                               scalar=bc[:, 3 * i + 1:3 * i + 2],
                                               in1=a[:], op0=AL.mult, op1=AL.add)
                nc.vector.tensor_scalar(out=a[:], in0=a[:],
                                        scalar1=bc[:, 3 * i + 2:3 * i + 3],
                                        scalar2=None, op0=AL.add)
                e = sp.tile([P, FCAM], f32, tag="e_i")
                nc.vector.tensor_tensor(out=e[:], in0=a[:], in1=d, op=AL.mult)
                nc.vector.tensor_scalar(out=e[:], in0=e[:],
                                        scalar1=bc[:, 9 + i:10 + i],
                                        scalar2=None, op0=AL.add)
                mid = (X / 2.0, Y / 2.0, Z / 2.0)[i]
                g = sp.tile([P, FCAM], f32, tag=f"g_{i}")
                nc.scalar.activation(g[:], e[:], ACT.Copy, bias=mid, scale=2.0)
                gs.append(g)
            gx, gy, gz = gs
            v = sp.tile([P, FCAM], f32, tag="v")
            nc.vector.tensor_scalar(out=v[:], in0=gx[:], scalar1=-1.0,
                                    scalar2=None, op0=AL.is_gt)
            nc.vector.scalar_tensor_tensor(out=v[:], in0=gx[:], scalar=float(X),
                                           in1=v[:], op0=AL.is_lt,
                                           op1=AL.logical_and)
            for gg, bound in ((gy, float(Y)), (gz, float(Z))):
                v2 = sp.tile([P, FCAM], f32, tag="v2")
                nc.vector.tensor_scalar(out=v2[:], in0=gg[:], scalar1=-1.0,
                                        scalar2=None, op0=AL.is_gt)
                nc.vector.scalar_tensor_tensor(out=v2[:], in0=gg[:], scalar=bound,
                                               in1=v2[:], op0=AL.is_lt,
                                               op1=AL.logical_and)
                nc.vector.tensor_tensor(out=v[:], in0=v[:], in1=v2[:],
                                        op=AL.logical_and)
            fx = sp.tile([P, FCAM], f32, tag="fx")
            fy = sp.tile([P, FCAM], f32, tag="fy")
            fz = sp.tile([P, FCAM], f32, tag="fz")
            _floor(nc, sp, fx[:], gx[:], FCAM)
            _floor(nc, sp, fy[:], gy[:], FCAM)
            _floor(nc, sp, fz[:], gz[:], FCAM)
            for ft in (fx, fy, fz):
                nc.vector.tensor_scalar(out=ft[:], in0=ft[:], scalar1=0.0,
                                        scalar2=255.0, op0=AL.max, op1=AL.min)
            lf = linf[:, cs]
            nc.vector.scalar_tensor_tensor(out=lf, in0=fz[:], scalar=float(X),
                                           in1=fx[:], op0=AL.mult, op1=AL.add)
            nc.vector.scalar_tensor_tensor(out=lf, in0=lf, scalar=float(Y),
                                           in1=fy[:], op0=AL.mult, op1=AL.add)
            # blend invalid -> INV :  lin = INV + v*(lin-INV)
            nc.vector.tensor_scalar(out=lf, in0=lf, scalar1=-INV,
                                    scalar2=None, op0=AL.add)
            nc.vector.tensor_tensor(out=lf, in0=lf, in1=v[:], op=AL.mult)
            nc.vector.tensor_scalar(out=lf, in0=lf, scalar1=INV,
                                    scalar2=None, op0=AL.add)
        # add per-chunk region offsets (invalid stays > BOUND)
        nc.vector.tensor_tensor(out=linf[:], in0=linf[:], in1=regoff[:],
                                op=AL.add)

        # ---- stage lin to DRAM: chunk-major [F,P] and partition-major [P,F]
        nc.sync.dma_start(lin_dram2[:, :], linf[:])
        NBLK = (F + P - 1) // P
        lin_t = pp.tile([P, NBLK, P], f32, tag="lin_t")
        for blk in range(NBLK):
            w = min(P, F - blk * P)
            ltp = psp.tile([P, P], f32, tag="tp")
            nc.tensor.transpose(out=ltp[:w, :], in_=linf[:, blk * P:blk * P + w],
                                identity=ident[:])
            nc.vector.tensor_copy(lin_t[:w, blk, :], ltp[:w, :])
            nc.sync.dma_start(lin_dram[blk * P:blk * P + w, :],
                              lin_t[:w, blk, :])

        sp_cm.__exit__(None, None, None)

        # ---- scatter: BF-chunk groups, one CCE-add call per group ----
        def group_body(gidx, alt):
            idrow = wp.tile([1, BF * P], f32, tag="idrow")
            nc.sync.dma_start(
                idrow[:], lin_dram[bass.ts(gidx, BF), :]
                .rearrange("a b -> (a b)")[None, :])
            col4 = wp.tile([P, BF], f32, tag="col4")
            nc.sync.dma_start(col4[:], lin_dram2[:, bass.ts(gidx, BF)])
            payc = wp.tile([P, BF, CG], GD, tag="payc")
            nc.scalar.dma_start(
                payc[:], feat[bass.ts(gidx, BF * P), :]
                .rearrange("(k p) c -> p k c", p=P))
            # widerow[p, j*128+f] = id of token f of chunk j  (outer product)
            wrow = psp.tile([P, BF * P], f32, tag="wrow")
            nc.tensor.matmul(wrow[:], lhsT=ones_col[:], rhs=idrow[:],
                             start=True, stop=True)
            eq = wp.tile([P, BF, P], EQD, tag="eq")
            for j in range(BF):
                nc.vector.tensor_tensor(
                    out=eq[:, j, :],
                    in0=col4[:, j:j + 1].to_broadcast([P, P]),
                    in1=wrow[:, j * P:(j + 1) * P], op=AL.is_equal)
            earl = wp.tile([P, BF, P], EQD, tag="earl")
            nc.vector.tensor_tensor(out=earl[:], in0=eq[:], in1=lowm[:],
                                    op=AL.mult)
            ecnt = wp.tile([P, BF], f32, tag="ecnt")
            nc.vector.tensor_reduce(out=ecnt[:], in_=earl[:],
                                    axis=mybir.AxisListType.X, op=AL.add)
            totp = psp.tile([P, BF * CG], f32, tag="totp")
            for j in range(BF):
                nc.tensor.matmul(totp[:, j * CG:(j + 1) * CG], lhsT=eq[:, j, :],
                                 rhs=payc[:, j, :], start=True, stop=True)
            tot = wp.tile([P, BF * CG], GD, tag="tot")
            nc.scalar.copy(tot[:], totp[:])
            # di = keep ? col : trash   (keep = first occurrence & valid)
            isval = wp.tile([P, BF], f32, tag="isval")
            nc.vector.tensor_scalar(out=isval[:], in0=col4[:], scalar1=INV,
                                    scalar2=None, op0=AL.is_lt)
            keep = wp.tile([P, BF], f32, tag="keep")
            nc.vector.scalar_tensor_tensor(out=keep[:], in0=ecnt[:], scalar=0.0,
                                           in1=isval[:], op0=AL.is_equal,
                                           op1=AL.logical_and)
            dif = wp.tile([P, BF], f32, tag="dif")
            nc.vector.tensor_tensor(out=dif[:], in0=col4[:], in1=trashc[:],
                                    op=AL.subtract)
            nc.vector.tensor_tensor(out=dif[:], in0=dif[:], in1=keep[:],
                                    op=AL.mult)
            nc.vector.tensor_tensor(out=dif[:], in0=dif[:], in1=trashc[:],
                                    op=AL.add)
            di = wp.tile([P, BF], i32, tag="di")
            nc.scalar.copy(di[:], dif[:])
            nc.gpsimd.indirect_dma_start(
                out=grids[alt],
                out_offset=bass.IndirectOffsetOnAxis(ap=di[:], axis=0),
                in_=tot[:], in_offset=None,
                compute_op=AL.add)

        def unrollable_body(iv0, unroll):
            for i in range(unroll):
                group_body(iv0 + i, i % NT)

        tc.For_i_unrolled_general(start=0, end=G, step=1,
                                  unrollable_body=unrollable_body,
                                  max_unroll=GUNROLL)

        # ---- merge regions + z-max + store ----
        wp_cm.__exit__(None, None, None)
        SL = X * Y
        HF = SL // P // 2  # half-slice free length (256)
        with tc.tile_pool(name="merge", bufs=2) as mp, \
             tc.tile_pool(name="mload", bufs=6 if GD == bf16 else 3) as lp:
            for h in range(2):
                acc = mp.tile([P, HF, CG], GD, tag="acc")
                for z in range(Z):
                    base = z * SL + h * (SL // 2)
                    sz = mp.tile([P, HF, CG], GD, tag="sz")
                    szin = None
                    for r in range(BF):
                        for t in range(NT):
                            rows = slice(r * NROWS + base,
                                         r * NROWS + base + SL // 2)
                            ta = lp.tile([P, HF, CG], GD, tag="ta")
                            eng = nc.sync if (r * NT + t) % 2 == 0 else nc.scalar
                            eng.dma_start(
                                ta[:],
                                grids[t][rows, :]
                                .rearrange("(p f) c -> p f c", p=P))
                            if szin is None:
                                szin = ta
                            else:
                                nc.vector.tensor_tensor(out=sz[:], in0=szin[:],
                                                        in1=ta[:], op=AL.add)
                                szin = sz
                    if z == 0:
                        nc.vector.tensor_copy(acc[:], szin[:])
                    else:
                        nc.vector.tensor_tensor(out=acc[:], in0=acc[:],
                                                in1=szin[:], op=AL.max)
                accf = mp.tile([P, HF, CG], f32, tag="accf")
                nc.vector.tensor_copy(accf[:], acc[:])
                orows = slice(h * (SL // 2), (h + 1) * (SL // 2))
                nc.sync.dma_start(
                    out[orows, :].rearrange("(p f) c -> p f c", p=P), accf[:])


def kernel(feat_maps, depths, K, E, T):
    global _prog_cache
    feat_maps = np.asarray(feat_maps, np.float32)
    depths = np.asarray(depths, np.float32)
    K = np.asarray(K, np.float32)
    E = np.asarray(E, np.float32)
    T = np.asarray(T, np.float32)

    if _prog_cache is None:
        _prog_cache = _build_program()
    nc = _prog_cache

    in_maps = []
    for core in range(8):
        b, cg = core // 4, core % 4
        ch = slice(cg * CG, (cg + 1) * CG)
        frows = np.concatenate([
            np.ascontiguousarray(
                feat_maps[b * N + n, ch].transpose(1, 2, 0).reshape(HW, CG)
                .reshape(P, FCAM, CG).transpose(1, 0, 2).reshape(HW, CG))
            for n in range(N)], axis=0).astype(NPDT)
        cams = slice(b * N, (b + 1) * N)
        in_maps.append({
            "feat": frows,
            "dep": np.ascontiguousarray(depths[b].reshape(NTOK)),
            "kmat": np.ascontiguousarray(K[cams]),
            "emat": np.ascontiguousarray(E[cams]),
            "tmat_t": np.ascontiguousarray(T[cams].transpose(0, 2, 1)),
        })

    _tr = bool(os.environ.get("KBEV_TRACE"))
    res = bass_utils.run_bass_kernel_spmd(
        nc, in_maps, core_ids=list(range(8)), trace=_tr,
        trace_cores=(list(range(8)) if os.environ.get("KBEV_TRACE_ALL")
                     else [0]) if _tr else None)
    global last_result
    last_result = res
    outp = np.zeros((1, B * C, X, Y), np.float32)
    for core in range(8):
        b, cg = core // 4, core % 4
        o = np.asarray(res.results[core]["out"]).reshape(X, Y, CG)
        outp[0, b * C + cg * CG:b * C + (cg + 1) * CG] = \
            o.transpose(2, 0, 1)[:, ::-1, ::-1]
    return outp


# revision 17
# speedup vs baseline: 1.0580x; 1.0580x over previous
"""BEVLiftNet Trainium2 kernel (V3: region-batched CCE-add scatter).

Sharding: 8 cores = 2 batches x 4 channel-groups (16 ch each).
Each core lifts all 4 cameras of its batch (depths -> voxel ids), then
scatter-adds 16-channel bf16 feature rows into DRAM grids via indirect
DMA with CCE-add.  Concurrent CCE RMWs to the same address lose
updates (measured), and the per-call gpsimd cost is ~1.7us regardless
of row count (measured), so calls are batched: one call = BF chunks of
128 tokens, chunk j of a call scattering into region j (disjoint
524288-row range) of one grid tensor.  Within-chunk duplicates are
pre-combined with an eq-matrix matmul; redundant/invalid descriptors
are dropped in hardware via the indirect-DMA bounds check (offsets
pushed past the bound by ecnt*DROPK).  Calls alternate between two
region-tensors so two WAW chains pipeline.  Cross-chunk duplicates
land in different regions and are summed by the final merge pass,
which adds all regions, max-reduces over Z, and stores [X*Y,16] f32;
the host reassembles the [1, B*C, X, Y] output.
"""

import os
import sys

sys.path.insert(0, "/opt/trn_rl_repo")

import ml_dtypes
import numpy as np

import concourse.bacc as bacc
import concourse.bass as bass
import concourse.mybir as mybir
import concourse.tile as tile
from concourse import bass_utils
from concourse.masks import make_identity

B, N, C, H, W = 2, 4, 64, 128, 352
X, Y, Z = 256, 256, 8
CG = 16           # channels per core
NCAM = 4          # cameras per core (one batch)
HW = H * W        # 45056 pixels per camera
NTOK = NCAM * HW  # 180224 tokens per core
P = 128
FCAM = HW // P    # 352 chunk-columns per camera
F = NCAM * FCAM   # 1408 chunk-columns total
NROWS = Z * X * Y          # 524288 voxel rows per region
f32 = mybir.dt.float32
bf16 = mybir.dt.bfloat16
i32 = mybir.dt.int32
AL = mybir.AluOpType
ACT = mybir.ActivationFunctionType

BF = int(os.environ.get("KBEV_BF", "1"))        # chunks (regions) per call
NT = int(os.environ.get("KBEV_NT", "4"))        # alternating grid tensors
GD = bf16 if os.environ.get("KBEV_GD", "f32") == "bf16" else f32
GUNROLL = int(os.environ.get("KBEV_UNROLL", "64"))  # groups per loop iter
G = F // BF                                     # scatter groups (calls)
GRID_ROWS = BF * NROWS + BF * P   # + per-(chunk,partition) trash rows
INV = float(BF * NROWS)   # invalid-token id marker (>= all valid ids)
NPDT = ml_dtypes.bfloat16 if GD == bf16 else np.float32
EQD = GD

_prog_cache = None


def _build_program():
    nc = bacc.Bacc("TRN2", target_bir_lowering=False, debug=False)
    feat = nc.dram_tensor("feat", [NTOK, CG], GD, kind="ExternalInput")
    dep = nc.dram_tensor("dep", [NTOK], f32, kind="ExternalInput")
    kmat = nc.dram_tensor("kmat", [NCAM, 3, 3], f32, kind="ExternalInput")
    emat = nc.dram_tensor("emat", [NCAM, 4, 4], f32, kind="ExternalInput")
    tmat_t = nc.dram_tensor("tmat_t", [NCAM, 4, 4], f32, kind="ExternalInput")
    out = nc.dram_tensor("out", [X * Y, CG], f32, kind="ExternalOutput")
    grids = [nc.dram_tensor(f"grid{t}", [GRID_ROWS, CG], GD, kind="Internal")
             for t in range(NT)]
    lin_dram = nc.dram_tensor("lin_dram", [F, P], f32, kind="Internal")
    lin_dram2 = nc.dram_tensor("lin_dram2", [P, F], f32, kind="Internal")
    with tile.TileContext(nc) as tc:
        _emit(tc, feat.ap(), dep.ap(), kmat.ap(), emat.ap(), tmat_t.ap(),
              out.ap(), [g.ap() for g in grids], lin_dram.ap(), lin_dram2.ap())
    nc.compile()
    return nc


def _floor(nc, wp, out_t, in_t, n):
    """out = floor(in), robust to any f32->i32 rounding mode."""
    ii = wp.tile([P, n], i32, tag="fl_i")
    nc.vector.tensor_copy(ii[:], in_t)
    ff = wp.tile([P, n], f32, tag="fl_f")
    nc.vector.tensor_copy(ff[:], ii[:])
    gt = wp.tile([P, n], f32, tag="fl_g")
    nc.vector.tensor_tensor(out=gt[:], in0=ff[:], in1=in_t, op=AL.is_gt)
    nc.vector.tensor_tensor(out=out_t, in0=ff[:], in1=gt[:], op=AL.subtract)


def _emit(tc, feat, dep, kmat, emat, tmat_t, out, grids, lin_dram, lin_dram2):
    nc = tc.nc

    with tc.tile_pool(name="persist", bufs=1) as pp, \
         tc.tile_pool(name="psum", bufs=2, space="PSUM") as psp:

        ident = pp.tile([P, P], f32, tag="ident")
        make_identity(nc, ident[:])
        # lowmask[p, k] = 1.0 where k < p, replicated BF times along free
        iot = pp.tile([P, P], i32, tag="iotpk")
        nc.gpsimd.iota(iot[:], pattern=[[1, P]], base=0, channel_multiplier=-1)
        lowm = pp.tile([P, BF, P], EQD, tag="lowmask")
        for j in range(BF):
            nc.vector.tensor_scalar(out=lowm[:, j, :], in0=iot[:], scalar1=0,
                                    scalar2=None, op0=AL.is_lt)
        ones_col = pp.tile([1, P], f32, tag="ones_col")
        nc.vector.memset(ones_col[:], 1.0)
        # trashc[p, j] = BF*NROWS + j*P + p  (unique in-call trash rows)
        tio = pp.tile([P, BF], i32, tag="tio")
        nc.gpsimd.iota(tio[:], pattern=[[P, BF]], base=BF * NROWS,
                       channel_multiplier=1)
        trashc = pp.tile([P, BF], f32, tag="trashc")
        nc.scalar.copy(trashc[:], tio[:])

        # ---- zero the grid tensors (HWDGE, overlaps with lift) ----
        with tc.tile_pool(name="zpool", bufs=1) as zp:
            ztile = zp.tile([P, 4096], GD, tag="zz")
            nc.vector.memset(ztile[:], 0.0)
            whole = GRID_ROWS * CG
            step = P * 4096
            for g in grids:
                gflat = g.rearrange("a c -> (a c)")
                starts = list(range(0, whole - step + 1, step))
                if starts[-1] + step < whole:
                    starts.append(whole - step)  # overlapped tail, re-zeroed
                for s in starts:
                    nc.sync.dma_start(
                        gflat[s:s + step].rearrange("(p m) -> p m", p=P),
                        ztile[:])

        # ---- pixel-constant tiles (shared by all cameras) ----
        wp_cm = tc.tile_pool(name="work", bufs=6)
        wp = wp_cm.__enter__()
        sp_cm = tc.tile_pool(name="setup", bufs=2)
        sp = sp_cm.__enter__()
        idl = pp.tile([P, FCAM], i32, tag="idl")
        nc.gpsimd.iota(idl[:], pattern=[[1, FCAM]], base=0, channel_multiplier=FCAM)
        idf = pp.tile([P, FCAM], f32, tag="idf")
        nc.scalar.copy(idf[:], idl[:])
        yf = pp.tile([P, FCAM], f32, tag="yf")
        xf = pp.tile([P, FCAM], f32, tag="xf")
        tmp = sp.tile([P, FCAM], f32, tag="t0")
        nc.scalar.activation(tmp[:], idf[:], ACT.Copy, bias=0.5 / W, scale=1.0 / W)
        _floor(nc, sp, yf[:], tmp[:], FCAM)
        nc.vector.scalar_tensor_tensor(out=xf[:], in0=yf[:], scalar=-float(W),
                                       in1=idf[:], op0=AL.mult, op1=AL.add)

        # region offset per chunk-column: regoff[p, f] = (f % BF) * NROWS
        fio = sp.tile([P, F], i32, tag="fio")
        nc.gpsimd.iota(fio[:], pattern=[[1, F]], base=0, channel_multiplier=0)
        fif = sp.tile([P, F], f32, tag="fif")
        nc.scalar.copy(fif[:], fio[:])
        fdiv = sp.tile([P, F], f32, tag="fdiv")
        nc.vector.tensor_scalar(out=fdiv[:], in0=fif[:], scalar1=1.0 / BF,
                                scalar2=None, op0=AL.mult)
        _floor(nc, sp, fdiv[:], fdiv[:], F)
        regoff = pp.tile([P, F], f32, tag="regoff")
        nc.vector.scalar_tensor_tensor(out=regoff[:], in0=fdiv[:],
                                       scalar=-float(BF), in1=fif[:],
                                       op0=AL.mult, op1=AL.add)
        nc.vector.tensor_scalar(out=regoff[:], in0=regoff[:],
                                scalar1=float(NROWS), scalar2=None, op0=AL.mult)

        # ---- per-camera geometry constants ----
        cam_consts = []
        for cam in range(NCAM):
            kc = pp.tile([3, 3], f32, tag=f"kc_{cam}")
            ec = pp.tile([4, 4], f32, tag=f"ec_{cam}")
            tmc = pp.tile([4, 4], f32, tag=f"tc_{cam}")
            nc.sync.dma_start(kc[:], kmat[cam])
            nc.sync.dma_start(ec[:], emat[cam])
            nc.sync.dma_start(tmc[:], tmat_t[cam])
            m4p = psp.tile([4, 4], f32, tag="smallp")
            nc.tensor.matmul(m4p[:], lhsT=tmc[:], rhs=ec[:],
                             start=True, stop=True)
            m4 = pp.tile([4, 4], f32, tag=f"m4_{cam}")
            nc.vector.tensor_copy(m4[:], m4p[:])
            m4tp = psp.tile([4, 4], f32, tag="smallp")
            nc.tensor.transpose(out=m4tp[:], in_=m4[:], identity=ident[:4, :4])
            m4t = pp.tile([4, 4], f32, tag=f"m4t_{cam}")
            nc.vector.tensor_copy(m4t[:], m4tp[:])
            n3p = psp.tile([3, 3], f32, tag="smallp")
            nc.tensor.matmul(n3p[:], lhsT=m4t[:3, :3], rhs=kc[:],
                             start=True, stop=True)
            n3 = pp.tile([3, 3], f32, tag=f"n3_{cam}")
            nc.vector.tensor_copy(n3[:], n3p[:])
            pk = pp.tile([1, 12], f32, tag=f"pk_{cam}")
            for i in range(3):
                nc.gpsimd.dma_start(pk[:, 3 * i:3 * i + 3], n3[i:i + 1, :])
                nc.gpsimd.dma_start(pk[:, 9 + i:10 + i], m4[i:i + 1, 3:4])
            bc = pp.tile([P, 12], f32, tag=f"bc_{cam}")
            nc.gpsimd.partition_broadcast(bc[:], pk[:])
            cam_consts.append(bc)

        # ---- load depths ----
        dtile = pp.tile([P, F], f32, tag="dtile")
        for cam in range(NCAM):
            cs = slice(cam * FCAM, (cam + 1) * FCAM)
            nc.sync.dma_start(
                dtile[:, cs],
                dep[cam * HW:(cam + 1) * HW].rearrange("(p f) -> p f", p=P))

        # ---- lift: voxel linear index per token ----
        linf = pp.tile([P, F], f32, tag="linf")
        nc.vector.memset(linf[:], INV)
        for cam in range(NCAM):
            cs = slice(cam * FCAM, (cam + 1) * FCAM)
            bc = cam_consts[cam]
            d = dtile[:, cs]
            gs = []
            for i in range(3):
                a = sp.tile([P, FCAM], f32, tag="a_i")
                nc.vector.tensor_scalar(out=a[:], in0=xf[:],
                                        scalar1=bc[:, 3 * i:3 * i + 1],
                                        scalar2=None, op0=AL.mult)
                nc.vector.scalar_tensor_tensor(out=a[:], in0=yf[:],
                                               scalar=bc[:, 3 * i + 1:3 * i + 2],
                                               in1=a[:], op0=AL.mult, op1=AL.add)
                nc.vector.tensor_scalar(out=a[:], in0=a[:],
                                        scalar1=bc[:, 3 * i + 2:3 * i + 3],
                                        scalar2=None, op0=AL.add)
                e = sp.tile([P, FCAM], f32, tag="e_i")
                nc.vector.tensor_tensor(out=e[:], in0=a[:], in1=d, op=AL.mult)
                nc.vector.tensor_scalar(out=e[:], in0=e[:],
                                        scalar1=bc[:, 9 + i:10 + i],
                                        scalar2=None, op0=AL.add)
                mid = (X / 2.0, Y / 2.0, Z / 2.0)[i]
                g = sp.tile([P, FCAM], f32, tag=f"g_{i}")
                nc.scalar.activation(g[:], e[:], ACT.Copy, bias=mid, scale=2.0)
                gs.append(g)
            gx, gy, gz = gs
            v = sp.tile([P, FCAM], f32, tag="v")
            nc.vector.tensor_scalar(out=v[:], in0=gx[:], scalar1=-1.0,
                                    scalar2=None, op0=AL.is_gt)
            nc.vector.scalar_tensor_tensor(out=v[:], in0=gx[:], scalar=float(X),
                                           in1=v[:], op0=AL.is_lt,
                                           op1=AL.logical_and)
            for gg, bound in ((gy, float(Y)), (gz, float(Z))):
                v2 = sp.tile([P, FCAM], f32, tag="v2")
                nc.vector.tensor_scalar(out=v2[:], in0=gg[:], scalar1=-1.0,
                                        scalar2=None, op0=AL.is_gt)
                nc.vector.scalar_tensor_tensor(out=v2[:], in0=gg[:], scalar=bound,
                                               in1=v2[:], op0=AL.is_lt,
                                               op1=AL.logical_and)
                nc.vector.tensor_tensor(out=v[:], in0=v[:], in1=v2[:],
                                        op=AL.logical_and)
            fx = sp.tile([P, FCAM], f32, tag="fx")
            fy = sp.tile([P, FCAM], f32, tag="fy")
            fz = sp.tile([P, FCAM], f32, tag="fz")
            _floor(nc, sp, fx[:], gx[:], FCAM)
            _floor(nc, sp, fy[:], gy[:], FCAM)
            _floor(nc, sp, fz[:], gz[:], FCAM)
            for ft in (fx, fy, fz):
                nc.vector.tensor_scalar(out=ft[:], in0=ft[:], scalar1=0.0,
                                        scalar2=255.0, op0=AL.max, op1=AL.min)
            lf = linf[:, cs]
            nc.vector.scalar_tensor_tensor(out=lf, in0=fz[:], scalar=float(X),
                                           in1=fx[:], op0=AL.mult, op1=AL.add)
            nc.vector.scalar_tensor_tensor(out=lf, in0=lf, scalar=float(Y),
                                           in1=fy[:], op0=AL.mult, op1=AL.add)
            # blend invalid -> INV :  lin = INV + v*(lin-INV)
            nc.vector.tensor_scalar(out=lf, in0=lf, scalar1=-INV,
                                    scalar2=None, op0=AL.add)
            nc.vector.tensor_tensor(out=lf, in0=lf, in1=v[:], op=AL.mult)
            nc.vector.tensor_scalar(out=lf, in0=lf, scalar1=INV,
                                    scalar2=None, op0=AL.add)
        # add per-chunk region offsets (invalid stays > BOUND)
        nc.vector.tensor_tensor(out=linf[:], in0=linf[:], in1=regoff[:],
                                op=AL.add)

        # ---- stage lin to DRAM: chunk-major [F,P] and partition-major [P,F]
        nc.sync.dma_start(lin_dram2[:, :], linf[:])
        NBLK = (F + P - 1) // P
        lin_t = pp.tile([P, NBLK, P], f32, tag="lin_t")
        for blk in range(NBLK):
            w = min(P, F - blk * P)
            ltp = psp.tile([P, P], f32, tag="tp")
            nc.tensor.transpose(out=ltp[:w, :], in_=linf[:, blk * P:blk * P + w],
                                identity=ident[:])
            nc.vector.tensor_copy(lin_t[:w, blk, :], ltp[:w, :])
            nc.sync.dma_start(lin_dram[blk * P:blk * P + w, :],
                              lin_t[:w, blk, :])

        sp_cm.__exit__(None, None, None)

        # ---- scatter: BF-chunk groups, one CCE-add call per group ----
        def group_body(gidx, alt):
            idrow = wp.tile([1, BF * P], f32, tag="idrow")
            nc.sync.dma_start(
                idrow[:], lin_dram[bass.ts(gidx, BF), :]
                .rearrange("a b -> (a b)")[None, :])
            col4 = wp.tile([P, BF], f32, tag="col4")
            nc.sync.dma_start(col4[:], lin_dram2[:, bass.ts(gidx, BF)])
            payc = wp.tile([P, BF, CG], GD, tag="payc")
            nc.scalar.dma_start(
                payc[:], feat[bass.ts(gidx, BF * P), :]
                .rearrange("(k p) c -> p k c", p=P))
            # widerow[p, j*128+f] = id of token f of chunk j  (outer product)
            wrow = psp.tile([P, BF * P], f32, tag="wrow")
            nc.tensor.matmul(wrow[:], lhsT=ones_col[:], rhs=idrow[:],
                             start=True, stop=True)
            eq = wp.tile([P, BF, P], EQD, tag="eq")
            for j in range(BF):
                nc.vector.tensor_tensor(
                    out=eq[:, j, :],
                    in0=col4[:, j:j + 1].to_broadcast([P, P]),
                    in1=wrow[:, j * P:(j + 1) * P], op=AL.is_equal)
            earl = wp.tile([P, BF, P], EQD, tag="earl")
            nc.vector.tensor_tensor(out=earl[:], in0=eq[:], in1=lowm[:],
                                    op=AL.mult)
            ecnt = wp.tile([P, BF], f32, tag="ecnt")
            nc.vector.tensor_reduce(out=ecnt[:], in_=earl[:],
                                    axis=mybir.AxisListType.X, op=AL.add)
            totp = psp.tile([P, BF * CG], f32, tag="totp")
            for j in range(BF):
                nc.tensor.matmul(totp[:, j * CG:(j + 1) * CG], lhsT=eq[:, j, :],
                                 rhs=payc[:, j, :], start=True, stop=True)
            tot = wp.tile([P, BF * CG], GD, tag="tot")
            nc.scalar.copy(tot[:], totp[:])
            # di = keep ? col : trash   (keep = first occurrence & valid)
            isval = wp.tile([P, BF], f32, tag="isval")
            nc.vector.tensor_scalar(out=isval[:], in0=col4[:], scalar1=INV,
                                    scalar2=None, op0=AL.is_lt)
            keep = wp.tile([P, BF], f32, tag="keep")
            nc.vector.scalar_tensor_tensor(out=keep[:], in0=ecnt[:], scalar=0.0,
                                           in1=isval[:], op0=AL.is_equal,
                                           op1=AL.logical_and)
            dif = wp.tile([P, BF], f32, tag="dif")
            nc.vector.tensor_tensor(out=dif[:], in0=col4[:], in1=trashc[:],
                                    op=AL.subtract)
            nc.vector.tensor_tensor(out=dif[:], in0=dif[:], in1=keep[:],
                                    op=AL.mult)
            nc.vector.tensor_tensor(out=dif[:], in0=dif[:], in1=trashc[:],
                                    op=AL.add)
            di = wp.tile([P, BF], i32, tag="di")
            nc.scalar.copy(di[:], dif[:])
            nc.gpsimd.indirect_dma_start(
                out=grids[alt],
                out_offset=bass.IndirectOffsetOnAxis(ap=di[:], axis=0),
                in_=tot[:], in_offset=None,
                compute_op=AL.add)

        def unrollable_body(iv0, unroll):
            for i in range(unroll):
                group_body(iv0 + i, i % NT)

        tc.For_i_unrolled_general(start=0, end=G, step=1,
                                  unrollable_body=unrollable_body,
                                  max_unroll=GUNROLL)

        # ---- merge regions + z-max + store ----
        wp_cm.__exit__(None, None, None)
        SL = X * Y
        HF = SL // P // 2  # half-slice free length (256)
        with tc.tile_pool(name="merge", bufs=2) as mp, \
             tc.tile_pool(name="mload", bufs=6 if GD == bf16 else 3) as lp:
            for h in range(2):
                acc = mp.tile([P, HF, CG], GD, tag="acc")
                for z in range(Z):
                    base = z * SL + h * (SL // 2)
                    sz = mp.tile([P, HF, CG], GD, tag="sz")
                    szin = None
                    for r in range(BF):
                        for t in range(NT):
                            rows = slice(r * NROWS + base,
                                         r * NROWS + base + SL // 2)
                            ta = lp.tile([P, HF, CG], GD, tag="ta")
                            eng = nc.sync if (r * NT + t) % 2 == 0 else nc.scalar
                            eng.dma_start(
                                ta[:],
                                grids[t][rows, :]
                                .rearrange("(p f) c -> p f c", p=P))
                            if szin is None:
                                szin = ta
                            else:
                                nc.vector.tensor_tensor(out=sz[:], in0=szin[:],
                                                        in1=ta[:], op=AL.add)
                                szin = sz
                    if z == 0:
                        nc.vector.tensor_copy(acc[:], szin[:])
                    else:
                        nc.vector.tensor_tensor(out=acc[:], in0=acc[:],
                                                in1=szin[:], op=AL.max)
                accf = mp.tile([P, HF, CG], f32, tag="accf")
                nc.vector.tensor_copy(accf[:], acc[:])
                orows = slice(h * (SL // 2), (h + 1) * (SL // 2))
                nc.sync.dma_start(
                    out[orows, :].rearrange("(p f) c -> p f c", p=P), accf[:])


def kernel(feat_maps, depths, K, E, T):
    global _prog_cache
    feat_maps = np.asarray(feat_maps, np.float32)
    depths = np.asarray(depths, np.float32)
    K = np.asarray(K, np.float32)
    E = np.asarray(E, np.float32)
    T = np.asarray(T, np.float32)

    if _prog_cache is None:
        _prog_cache = _build_program()
    nc = _prog_cache

    in_maps = []
    for core in range(8):
        b, cg = core // 4, core % 4
        ch = slice(cg * CG, (cg + 1) * CG)
        frows = np.concatenate([
            np.ascontiguousarray(
                feat_maps[b * N + n, ch].transpose(1, 2, 0).reshape(HW, CG)
                .reshape(P, FCAM, CG).transpose(1, 0, 2).reshape(HW, CG))
            for n in range(N)], axis=0).astype(NPDT)
        cams = slice(b * N, (b + 1) * N)
        in_maps.append({
            "feat": frows,
            "dep": np.ascontiguousarray(depths[b].reshape(NTOK)),
            "kmat": np.ascontiguousarray(K[cams]),
            "emat": np.ascontiguousarray(E[cams]),
            "tmat_t": np.ascontiguousarray(T[cams].transpose(0, 2, 1)),
        })

    _tr = bool(os.environ.get("KBEV_TRACE"))
    res = bass_utils.run_bass_kernel_spmd(
        nc, in_maps, core_ids=list(range(8)), trace=_tr,
        trace_cores=(list(range(8)) if os.environ.get("KBEV_TRACE_ALL")
                     else [0]) if _tr else None)
    global last_result
    last_result = res
    outp = np.zeros((1, B * C, X, Y), np.float32)
    for core in range(8):
        b, cg = core // 4, core % 4
        o = np.asarray(res.results[core]["out"]).reshape(X, Y, CG)
        outp[0, b * C + cg * CG:b * C + (cg + 1) * CG] = \
            o.transpose(2, 0, 1)[:, ::-1, ::-1]
    return outp


# revision 19
# speedup vs baseline: 1.0664x; 1.0080x over previous
"""BEVLiftNet Trainium2 kernel (V3: direct CCE-add scatter chains).

Sharding: 8 cores = 2 batches x 4 channel-groups (16 ch each).
Each core lifts all 4 cameras of its batch (depths -> voxel ids), then
scatter-adds 16-channel f32 feature rows into NT round-robin DRAM grid
tensors via indirect DMA with CCE-add (compute_op=add), one call per
128-token chunk ([P,1] offsets -- the only offset shape the INDIRECT1D
ucode pairs correctly with payload; measured).  Concurrent CCE RMWs to
the same address lose updates (measured), so within-chunk duplicate
rows are pre-combined with an eq-matrix matmul (built against a PE
outer-product broadcast of the chunk ids, no per-chunk transpose) and
redirected to per-partition-unique trash rows past the voxel range.
Chunks round-robin over NT grid tensors so NT WAW chains pipeline and
gpsimd issues calls back-to-back (~1.7us fixed cost each).  A final
pass sums the grids, max-reduces over Z, and stores [X*Y, 16] f32; the
host reassembles the [1, B*C, X, Y] output.

Tuning knobs (env): KBEV_BF (chunks/call; >1 requires multi-column
offset support the current ucode lacks), KBEV_NT (grid tensors),
KBEV_GD (grid dtype), KBEV_UNROLL (chunks per hw-loop iteration).
"""

import os
import sys

sys.path.insert(0, "/opt/trn_rl_repo")

import ml_dtypes
import numpy as np

import concourse.bacc as bacc
import concourse.bass as bass
import concourse.mybir as mybir
import concourse.tile as tile
from concourse import bass_utils
from concourse.masks import make_identity

B, N, C, H, W = 2, 4, 64, 128, 352
X, Y, Z = 256, 256, 8
CG = 16           # channels per core
NCAM = 4          # cameras per core (one batch)
HW = H * W        # 45056 pixels per camera
NTOK = NCAM * HW  # 180224 tokens per core
P = 128
FCAM = HW // P    # 352 chunk-columns per camera
F = NCAM * FCAM   # 1408 chunk-columns total
NROWS = Z * X * Y          # 524288 voxel rows per region
f32 = mybir.dt.float32
bf16 = mybir.dt.bfloat16
i32 = mybir.dt.int32
AL = mybir.AluOpType
ACT = mybir.ActivationFunctionType

BF = int(os.environ.get("KBEV_BF", "1"))        # chunks (regions) per call
NT = int(os.environ.get("KBEV_NT", "4"))        # alternating grid tensors
GD = bf16 if os.environ.get("KBEV_GD", "f32") == "bf16" else f32
GUNROLL = int(os.environ.get("KBEV_UNROLL", "64"))  # groups per loop iter
G = F // BF                                     # scatter groups (calls)
GRID_ROWS = BF * NROWS + BF * P   # + per-(chunk,partition) trash rows
INV = float(BF * NROWS)   # invalid-token id marker (>= all valid ids)
NPDT = ml_dtypes.bfloat16 if GD == bf16 else np.float32
EQD = GD

_prog_cache = None


def _build_program():
    nc = bacc.Bacc("TRN2", target_bir_lowering=False, debug=False)
    feat = nc.dram_tensor("feat", [NTOK, CG], GD, kind="ExternalInput")
    dep = nc.dram_tensor("dep", [NTOK], f32, kind="ExternalInput")
    kmat = nc.dram_tensor("kmat", [NCAM, 3, 3], f32, kind="ExternalInput")
    emat = nc.dram_tensor("emat", [NCAM, 4, 4], f32, kind="ExternalInput")
    tmat_t = nc.dram_tensor("tmat_t", [NCAM, 4, 4], f32, kind="ExternalInput")
    out = nc.dram_tensor("out", [X * Y, CG], f32, kind="ExternalOutput")
    grids = [nc.dram_tensor(f"grid{t}", [GRID_ROWS, CG], GD, kind="Internal")
             for t in range(NT)]
    lin_dram = nc.dram_tensor("lin_dram", [F, P], f32, kind="Internal")
    lin_dram2 = nc.dram_tensor("lin_dram2", [P, F], f32, kind="Internal")
    with tile.TileContext(nc) as tc:
        _emit(tc, feat.ap(), dep.ap(), kmat.ap(), emat.ap(), tmat_t.ap(),
              out.ap(), [g.ap() for g in grids], lin_dram.ap(), lin_dram2.ap())
    nc.compile()
    return nc


def _floor(nc, wp, out_t, in_t, n):
    """out = floor(in), robust to any f32->i32 rounding mode."""
    ii = wp.tile([P, n], i32, tag="fl_i")
    nc.vector.tensor_copy(ii[:], in_t)
    ff = wp.tile([P, n], f32, tag="fl_f")
    nc.vector.tensor_copy(ff[:], ii[:])
    gt = wp.tile([P, n], f32, tag="fl_g")
    nc.vector.tensor_tensor(out=gt[:], in0=ff[:], in1=in_t, op=AL.is_gt)
    nc.vector.tensor_tensor(out=out_t, in0=ff[:], in1=gt[:], op=AL.subtract)


def _emit(tc, feat, dep, kmat, emat, tmat_t, out, grids, lin_dram, lin_dram2):
    nc = tc.nc

    with tc.tile_pool(name="persist", bufs=1) as pp, \
         tc.tile_pool(name="psum", bufs=2, space="PSUM") as psp:

        ident = pp.tile([P, P], f32, tag="ident")
        make_identity(nc, ident[:])
        # lowmask[p, k] = 1.0 where k < p, replicated BF times along free
        iot = pp.tile([P, P], i32, tag="iotpk")
        nc.gpsimd.iota(iot[:], pattern=[[1, P]], base=0, channel_multiplier=-1)
        lowm = pp.tile([P, BF, P], EQD, tag="lowmask")
        for j in range(BF):
            nc.vector.tensor_scalar(out=lowm[:, j, :], in0=iot[:], scalar1=0,
                                    scalar2=None, op0=AL.is_lt)
        ones_col = pp.tile([1, P], f32, tag="ones_col")
        nc.vector.memset(ones_col[:], 1.0)
        # trashc[p, j] = BF*NROWS + j*P + p  (unique in-call trash rows)
        tio = pp.tile([P, BF], i32, tag="tio")
        nc.gpsimd.iota(tio[:], pattern=[[P, BF]], base=BF * NROWS,
                       channel_multiplier=1)
        trashc = pp.tile([P, BF], f32, tag="trashc")
        nc.scalar.copy(trashc[:], tio[:])

        # ---- zero the grid tensors (HWDGE, overlaps with lift) ----
        with tc.tile_pool(name="zpool", bufs=1) as zp:
            ztile = zp.tile([P, 4096], GD, tag="zz")
            nc.vector.memset(ztile[:], 0.0)
            whole = GRID_ROWS * CG
            step = P * 4096
            for g in grids:
                gflat = g.rearrange("a c -> (a c)")
                starts = list(range(0, whole - step + 1, step))
                if starts[-1] + step < whole:
                    starts.append(whole - step)  # overlapped tail, re-zeroed
                for s in starts:
                    nc.sync.dma_start(
                        gflat[s:s + step].rearrange("(p m) -> p m", p=P),
                        ztile[:])

        # ---- pixel-constant tiles (shared by all cameras) ----
        wp_cm = tc.tile_pool(name="work", bufs=6)
        wp = wp_cm.__enter__()
        sp_cm = tc.tile_pool(name="setup", bufs=2)
        sp = sp_cm.__enter__()
        idl = pp.tile([P, FCAM], i32, tag="idl")
        nc.gpsimd.iota(idl[:], pattern=[[1, FCAM]], base=0, channel_multiplier=FCAM)
        idf = pp.tile([P, FCAM], f32, tag="idf")
        nc.scalar.copy(idf[:], idl[:])
        yf = pp.tile([P, FCAM], f32, tag="yf")
        xf = pp.tile([P, FCAM], f32, tag="xf")
        tmp = sp.tile([P, FCAM], f32, tag="t0")
        nc.scalar.activation(tmp[:], idf[:], ACT.Copy, bias=0.5 / W, scale=1.0 / W)
        _floor(nc, sp, yf[:], tmp[:], FCAM)
        nc.vector.scalar_tensor_tensor(out=xf[:], in0=yf[:], scalar=-float(W),
                                       in1=idf[:], op0=AL.mult, op1=AL.add)

        # region offset per chunk-column: regoff[p, f] = (f % BF) * NROWS
        fio = sp.tile([P, F], i32, tag="fio")
        nc.gpsimd.iota(fio[:], pattern=[[1, F]], base=0, channel_multiplier=0)
        fif = sp.tile([P, F], f32, tag="fif")
        nc.scalar.copy(fif[:], fio[:])
        fdiv = sp.tile([P, F], f32, tag="fdiv")
        nc.vector.tensor_scalar(out=fdiv[:], in0=fif[:], scalar1=1.0 / BF,
                                scalar2=None, op0=AL.mult)
        _floor(nc, sp, fdiv[:], fdiv[:], F)
        regoff = pp.tile([P, F], f32, tag="regoff")
        nc.vector.scalar_tensor_tensor(out=regoff[:], in0=fdiv[:],
                                       scalar=-float(BF), in1=fif[:],
                                       op0=AL.mult, op1=AL.add)
        nc.vector.tensor_scalar(out=regoff[:], in0=regoff[:],
                                scalar1=float(NROWS), scalar2=None, op0=AL.mult)

        # ---- per-camera geometry constants ----
        cam_consts = []
        for cam in range(NCAM):
            kc = pp.tile([3, 3], f32, tag=f"kc_{cam}")
            ec = pp.tile([4, 4], f32, tag=f"ec_{cam}")
            tmc = pp.tile([4, 4], f32, tag=f"tc_{cam}")
            nc.sync.dma_start(kc[:], kmat[cam])
            nc.sync.dma_start(ec[:], emat[cam])
            nc.sync.dma_start(tmc[:], tmat_t[cam])
            m4p = psp.tile([4, 4], f32, tag="smallp")
            nc.tensor.matmul(m4p[:], lhsT=tmc[:], rhs=ec[:],
                             start=True, stop=True)
            m4 = pp.tile([4, 4], f32, tag=f"m4_{cam}")
            nc.vector.tensor_copy(m4[:], m4p[:])
            m4tp = psp.tile([4, 4], f32, tag="smallp")
            nc.tensor.transpose(out=m4tp[:], in_=m4[:], identity=ident[:4, :4])
            m4t = pp.tile([4, 4], f32, tag=f"m4t_{cam}")
            nc.vector.tensor_copy(m4t[:], m4tp[:])
            n3p = psp.tile([3, 3], f32, tag="smallp")
            nc.tensor.matmul(n3p[:], lhsT=m4t[:3, :3], rhs=kc[:],
                             start=True, stop=True)
            n3 = pp.tile([3, 3], f32, tag=f"n3_{cam}")
            nc.vector.tensor_copy(n3[:], n3p[:])
            pk = pp.tile([1, 12], f32, tag=f"pk_{cam}")
            for i in range(3):
                nc.gpsimd.dma_start(pk[:, 3 * i:3 * i + 3], n3[i:i + 1, :])
                nc.gpsimd.dma_start(pk[:, 9 + i:10 + i], m4[i:i + 1, 3:4])
            bc = pp.tile([P, 12], f32, tag=f"bc_{cam}")
            nc.gpsimd.partition_broadcast(bc[:], pk[:])
            cam_consts.append(bc)

        # ---- load depths ----
        dtile = pp.tile([P, F], f32, tag="dtile")
        for cam in range(NCAM):
            cs = slice(cam * FCAM, (cam + 1) * FCAM)
            nc.sync.dma_start(
                dtile[:, cs],
                dep[cam * HW:(cam + 1) * HW].rearrange("(p f) -> p f", p=P))

        # ---- lift: voxel linear index per token ----
        linf = pp.tile([P, F], f32, tag="linf")
        nc.vector.memset(linf[:], INV)
        for cam in range(NCAM):
            cs = slice(cam * FCAM, (cam + 1) * FCAM)
            bc = cam_consts[cam]
            d = dtile[:, cs]
            gs = []
            for i in range(3):
                a = sp.tile([P, FCAM], f32, tag="a_i")
                nc.vector.tensor_scalar(out=a[:], in0=xf[:],
                                        scalar1=bc[:, 3 * i:3 * i + 1],
                                        scalar2=None, op0=AL.mult)
                nc.vector.scalar_tensor_tensor(out=a[:], in0=yf[:],
                                               scalar=bc[:, 3 * i + 1:3 * i + 2],
                                               in1=a[:], op0=AL.mult, op1=AL.add)
                nc.vector.tensor_scalar(out=a[:], in0=a[:],
                                        scalar1=bc[:, 3 * i + 2:3 * i + 3],
                                        scalar2=None, op0=AL.add)
                e = sp.tile([P, FCAM], f32, tag="e_i")
                nc.vector.tensor_tensor(out=e[:], in0=a[:], in1=d, op=AL.mult)
                nc.vector.tensor_scalar(out=e[:], in0=e[:],
                                        scalar1=bc[:, 9 + i:10 + i],
                                        scalar2=None, op0=AL.add)
                mid = (X / 2.0, Y / 2.0, Z / 2.0)[i]
                g = sp.tile([P, FCAM], f32, tag=f"g_{i}")
                nc.scalar.activation(g[:], e[:], ACT.Copy, bias=mid, scale=2.0)
                gs.append(g)
            gx, gy, gz = gs
            v = sp.tile([P, FCAM], f32, tag="v")
            nc.vector.tensor_scalar(out=v[:], in0=gx[:], scalar1=-1.0,
                                    scalar2=None, op0=AL.is_gt)
            nc.vector.scalar_tensor_tensor(out=v[:], in0=gx[:], scalar=float(X),
                                           in1=v[:], op0=AL.is_lt,
                                           op1=AL.logical_and)
            for gg, bound in ((gy, float(Y)), (gz, float(Z))):
                v2 = sp.tile([P, FCAM], f32, tag="v2")
                nc.vector.tensor_scalar(out=v2[:], in0=gg[:], scalar1=-1.0,
                                        scalar2=None, op0=AL.is_gt)
                nc.vector.scalar_tensor_tensor(out=v2[:], in0=gg[:], scalar=bound,
                                               in1=v2[:], op0=AL.is_lt,
                                               op1=AL.logical_and)
                nc.vector.tensor_tensor(out=v[:], in0=v[:], in1=v2[:],
                                        op=AL.logical_and)
            fx = sp.tile([P, FCAM], f32, tag="fx")
            fy = sp.tile([P, FCAM], f32, tag="fy")
            fz = sp.tile([P, FCAM], f32, tag="fz")
            _floor(nc, sp, fx[:], gx[:], FCAM)
            _floor(nc, sp, fy[:], gy[:], FCAM)
            _floor(nc, sp, fz[:], gz[:], FCAM)
            for ft in (fx, fy, fz):
                nc.vector.tensor_scalar(out=ft[:], in0=ft[:], scalar1=0.0,
                                        scalar2=255.0, op0=AL.max, op1=AL.min)
            lf = linf[:, cs]
            nc.vector.scalar_tensor_tensor(out=lf, in0=fz[:], scalar=float(X),
                                           in1=fx[:], op0=AL.mult, op1=AL.add)
            nc.vector.scalar_tensor_tensor(out=lf, in0=lf, scalar=float(Y),
                                           in1=fy[:], op0=AL.mult, op1=AL.add)
            # blend invalid -> INV :  lin = INV + v*(lin-INV)
            nc.vector.tensor_scalar(out=lf, in0=lf, scalar1=-INV,
                                    scalar2=None, op0=AL.add)
            nc.vector.tensor_tensor(out=lf, in0=lf, in1=v[:], op=AL.mult)
            nc.vector.tensor_scalar(out=lf, in0=lf, scalar1=INV,
                                    scalar2=None, op0=AL.add)
        # add per-chunk region offsets (invalid stays > BOUND)
        nc.vector.tensor_tensor(out=linf[:], in0=linf[:], in1=regoff[:],
                                op=AL.add)

        # ---- stage lin to DRAM: chunk-major [F,P] and partition-major [P,F]
        nc.sync.dma_start(lin_dram2[:, :], linf[:])
        NBLK = (F + P - 1) // P
        lin_t = pp.tile([P, NBLK, P], f32, tag="lin_t")
        for blk in range(NBLK):
            w = min(P, F - blk * P)
            ltp = psp.tile([P, P], f32, tag="tp")
            nc.tensor.transpose(out=ltp[:w, :], in_=linf[:, blk * P:blk * P + w],
                                identity=ident[:])
            nc.vector.tensor_copy(lin_t[:w, blk, :], ltp[:w, :])
            nc.sync.dma_start(lin_dram[blk * P:blk * P + w, :],
                              lin_t[:w, blk, :])

        sp_cm.__exit__(None, None, None)

        # ---- scatter: BF-chunk groups, one CCE-add call per group ----
        def group_body(gidx, alt):
            col4 = wp.tile([P, BF], f32, tag="col4")
            nc.sync.dma_start(col4[:], lin_dram2[:, bass.ts(gidx, BF)])
            payc = wp.tile([P, BF, CG], GD, tag="payc")
            nc.scalar.dma_start(
                payc[:], feat[bass.ts(gidx, BF * P), :]
                .rearrange("(k p) c -> p k c", p=P))
            # widerow[p, j*128+f] = id of token f of chunk j
            wrow = psp.tile([P, BF * P], f32, tag="wrow")
            if BF == 1:
                # transpose of the broadcast column == partition-broadcast row
                nc.tensor.transpose(out=wrow[:],
                                    in_=col4[:, 0:1].to_broadcast([P, P]),
                                    identity=ident[:])
            else:
                idrow = wp.tile([1, BF * P], f32, tag="idrow")
                nc.sync.dma_start(
                    idrow[:], lin_dram[bass.ts(gidx, BF), :]
                    .rearrange("a b -> (a b)")[None, :])
                nc.tensor.matmul(wrow[:], lhsT=ones_col[:], rhs=idrow[:],
                                 start=True, stop=True)
            eq = wp.tile([P, BF, P], EQD, tag="eq")
            for j in range(BF):
                nc.vector.tensor_tensor(
                    out=eq[:, j, :],
                    in0=col4[:, j:j + 1].to_broadcast([P, P]),
                    in1=wrow[:, j * P:(j + 1) * P], op=AL.is_equal)
            earl = wp.tile([P, BF, P], EQD, tag="earl")
            nc.vector.tensor_tensor(out=earl[:], in0=eq[:], in1=lowm[:],
                                    op=AL.mult)
            ecnt = wp.tile([P, BF], f32, tag="ecnt")
            nc.vector.tensor_reduce(out=ecnt[:], in_=earl[:],
                                    axis=mybir.AxisListType.X, op=AL.add)
            totp = psp.tile([P, BF * CG], f32, tag="totp")
            for j in range(BF):
                nc.tensor.matmul(totp[:, j * CG:(j + 1) * CG], lhsT=eq[:, j, :],
                                 rhs=payc[:, j, :], start=True, stop=True)
            tot = wp.tile([P, BF * CG], GD, tag="tot")
            nc.scalar.copy(tot[:], totp[:])
            # di = keep ? col : trash   (keep = first occurrence & valid)
            isval = wp.tile([P, BF], f32, tag="isval")
            nc.vector.tensor_scalar(out=isval[:], in0=col4[:], scalar1=INV,
                                    scalar2=None, op0=AL.is_lt)
            keep = wp.tile([P, BF], f32, tag="keep")
            nc.vector.scalar_tensor_tensor(out=keep[:], in0=ecnt[:], scalar=0.0,
                                           in1=isval[:], op0=AL.is_equal,
                                           op1=AL.logical_and)
            dif = wp.tile([P, BF], f32, tag="dif")
            nc.vector.tensor_tensor(out=dif[:], in0=col4[:], in1=trashc[:],
                                    op=AL.subtract)
            nc.vector.tensor_tensor(out=dif[:], in0=dif[:], in1=keep[:],
                                    op=AL.mult)
            nc.vector.tensor_tensor(out=dif[:], in0=dif[:], in1=trashc[:],
                                    op=AL.add)
            di = wp.tile([P, BF], i32, tag="di")
            nc.vector.tensor_copy(di[:], dif[:])
            nc.gpsimd.indirect_dma_start(
                out=grids[alt],
                out_offset=bass.IndirectOffsetOnAxis(ap=di[:], axis=0),
                in_=tot[:], in_offset=None,
                compute_op=AL.add)

        def unrollable_body(iv0, unroll):
            for i in range(unroll):
                group_body(iv0 + i, i % NT)

        tc.For_i_unrolled_general(start=0, end=G, step=1,
                                  unrollable_body=unrollable_body,
                                  max_unroll=GUNROLL)

        # ---- merge regions + z-max + store ----
        wp_cm.__exit__(None, None, None)
        SL = X * Y
        HF = SL // P // 2  # half-slice free length (256)
        with tc.tile_pool(name="merge", bufs=2) as mp, \
             tc.tile_pool(name="mload", bufs=6 if GD == bf16 else 3) as lp:
            for h in range(2):
                acc = mp.tile([P, HF, CG], GD, tag="acc")
                for z in range(Z):
                    base = z * SL + h * (SL // 2)
                    sz = mp.tile([P, HF, CG], GD, tag="sz")
                    szin = None
                    for r in range(BF):
                        for t in range(NT):
                            rows = slice(r * NROWS + base,
                                         r * NROWS + base + SL // 2)
                            ta = lp.tile([P, HF, CG], GD, tag="ta")
                            eng = nc.sync if (r * NT + t) % 2 == 0 else nc.scalar
                            eng.dma_start(
                                ta[:],
                                grids[t][rows, :]
                                .rearrange("(p f) c -> p f c", p=P))
                            if szin is None:
                                szin = ta
                            else:
                                nc.vector.tensor_tensor(out=sz[:], in0=szin[:],
                                                        in1=ta[:], op=AL.add)
                                szin = sz
                    if z == 0:
                        nc.vector.tensor_copy(acc[:], szin[:])
                    else:
                        nc.vector.tensor_tensor(out=acc[:], in0=acc[:],
                                                in1=szin[:], op=AL.max)
                accf = mp.tile([P, HF, CG], f32, tag="accf")
                nc.vector.tensor_copy(accf[:], acc[:])
                orows = slice(h * (SL // 2), (h + 1) * (SL // 2))
                nc.sync.dma_start(
                    out[orows, :].rearrange("(p f) c -> p f c", p=P), accf[:])


def kernel(feat_maps, depths, K, E, T):
    global _prog_cache
    feat_maps = np.asarray(feat_maps, np.float32)
    depths = np.asarray(depths, np.float32)
    K = np.asarray(K, np.float32)
    E = np.asarray(E, np.float32)
    T = np.asarray(T, np.float32)

    if _prog_cache is None:
        _prog_cache = _build_program()
    nc = _prog_cache

    in_maps = []
    for core in range(8):
        b, cg = core // 4, core % 4
        ch = slice(cg * CG, (cg + 1) * CG)
        frows = np.concatenate([
            np.ascontiguousarray(
                feat_maps[b * N + n, ch].transpose(1, 2, 0).reshape(HW, CG)
                .reshape(P, FCAM, CG).transpose(1, 0, 2).reshape(HW, CG))
            for n in range(N)], axis=0).astype(NPDT)
        cams = slice(b * N, (b + 1) * N)
        in_maps.append({
            "feat": frows,
            "dep": np.ascontiguousarray(depths[b].reshape(NTOK)),
            "kmat": np.ascontiguousarray(K[cams]),
            "emat": np.ascontiguousarray(E[cams]),
            "tmat_t": np.ascontiguousarray(T[cams].transpose(0, 2, 1)),
        })

    _tr = bool(os.environ.get("KBEV_TRACE"))
    res = bass_utils.run_bass_kernel_spmd(
        nc, in_maps, core_ids=list(range(8)), trace=_tr,
        trace_cores=(list(range(8)) if os.environ.get("KBEV_TRACE_ALL")
                     else [0]) if _tr else None)
    global last_result
    last_result = res
    outp = np.zeros((1, B * C, X, Y), np.float32)
    for core in range(8):
        b, cg = core // 4, core % 4
        o = np.asarray(res.results[core]["out"]).reshape(X, Y, CG)
        outp[0, b * C + cg * CG:b * C + (cg + 1) * CG] = \
            o.transpose(2, 0, 1)[:, ::-1, ::-1]
    return outp


# revision 21
# speedup vs baseline: 1.1086x; 1.0396x over previous
"""BEVLiftNet Trainium2 kernel (V3: direct CCE-add scatter chains).

Sharding: 8 cores = 2 batches x 4 channel-groups (16 ch each).
Each core lifts all 4 cameras of its batch (depths -> voxel ids), then
scatter-adds 16-channel f32 feature rows into NT round-robin DRAM grid
tensors via indirect DMA with CCE-add (compute_op=add), one call per
128-token chunk ([P,1] offsets -- the only offset shape the INDIRECT1D
ucode pairs correctly with payload; measured).  Concurrent CCE RMWs to
the same address lose updates (measured), so within-chunk duplicate
rows are pre-combined with an eq-matrix matmul (built against a PE
outer-product broadcast of the chunk ids, no per-chunk transpose) and
redirected to per-partition-unique trash rows past the voxel range.
Chunks round-robin over NT grid tensors so NT WAW chains pipeline and
gpsimd issues calls back-to-back (~1.7us fixed cost each).  A final
pass sums the grids, max-reduces over Z, and stores [X*Y, 16] f32; the
host reassembles the [1, B*C, X, Y] output.

Tuning knobs (env): KBEV_BF (chunks/call; >1 requires multi-column
offset support the current ucode lacks), KBEV_NT (grid tensors),
KBEV_GD (grid dtype), KBEV_UNROLL (chunks per hw-loop iteration).
"""

import os
import sys

sys.path.insert(0, "/opt/trn_rl_repo")

import ml_dtypes
import numpy as np

import concourse.bacc as bacc
import concourse.bass as bass
import concourse.mybir as mybir
import concourse.tile as tile
from concourse import bass_utils
from concourse.masks import make_identity

B, N, C, H, W = 2, 4, 64, 128, 352
X, Y, Z = 256, 256, 8
CG = 16           # channels per core
NCAM = 4          # cameras per core (one batch)
HW = H * W        # 45056 pixels per camera
NTOK = NCAM * HW  # 180224 tokens per core
P = 128
FCAM = HW // P    # 352 chunk-columns per camera
F = NCAM * FCAM   # 1408 chunk-columns total
NROWS = Z * X * Y          # 524288 voxel rows per region
f32 = mybir.dt.float32
bf16 = mybir.dt.bfloat16
i32 = mybir.dt.int32
AL = mybir.AluOpType
ACT = mybir.ActivationFunctionType

BF = int(os.environ.get("KBEV_BF", "1"))        # chunks (regions) per call
NT = int(os.environ.get("KBEV_NT", "4"))        # alternating grid tensors
GD = bf16 if os.environ.get("KBEV_GD", "f32") == "bf16" else f32
GUNROLL = int(os.environ.get("KBEV_UNROLL", "128"))  # groups per loop iter
G = F // BF                                     # scatter groups (calls)
GRID_ROWS = BF * NROWS + BF * P   # + per-(chunk,partition) trash rows
INV = float(BF * NROWS)   # invalid-token id marker (>= all valid ids)
NPDT = ml_dtypes.bfloat16 if GD == bf16 else np.float32
EQD = GD

_prog_cache = None


def _build_program():
    nc = bacc.Bacc("TRN2", target_bir_lowering=False, debug=False)
    feat = nc.dram_tensor("feat", [NTOK, CG], GD, kind="ExternalInput")
    dep = nc.dram_tensor("dep", [NTOK], f32, kind="ExternalInput")
    kmat = nc.dram_tensor("kmat", [NCAM, 3, 3], f32, kind="ExternalInput")
    emat = nc.dram_tensor("emat", [NCAM, 4, 4], f32, kind="ExternalInput")
    tmat_t = nc.dram_tensor("tmat_t", [NCAM, 4, 4], f32, kind="ExternalInput")
    out = nc.dram_tensor("out", [X * Y, CG], f32, kind="ExternalOutput")
    grids = [nc.dram_tensor(f"grid{t}", [GRID_ROWS, CG], GD, kind="Internal")
             for t in range(NT)]
    lin_dram = nc.dram_tensor("lin_dram", [F, P], f32, kind="Internal")
    lin_dram2 = nc.dram_tensor("lin_dram2", [P, F], f32, kind="Internal")
    with tile.TileContext(nc) as tc:
        _emit(tc, feat.ap(), dep.ap(), kmat.ap(), emat.ap(), tmat_t.ap(),
              out.ap(), [g.ap() for g in grids], lin_dram.ap(), lin_dram2.ap())
    nc.compile()
    return nc


def _floor(nc, wp, out_t, in_t, n):
    """out = floor(in), robust to any f32->i32 rounding mode."""
    ii = wp.tile([P, n], i32, tag="fl_i")
    nc.vector.tensor_copy(ii[:], in_t)
    ff = wp.tile([P, n], f32, tag="fl_f")
    nc.vector.tensor_copy(ff[:], ii[:])
    gt = wp.tile([P, n], f32, tag="fl_g")
    nc.vector.tensor_tensor(out=gt[:], in0=ff[:], in1=in_t, op=AL.is_gt)
    nc.vector.tensor_tensor(out=out_t, in0=ff[:], in1=gt[:], op=AL.subtract)


def _emit(tc, feat, dep, kmat, emat, tmat_t, out, grids, lin_dram, lin_dram2):
    nc = tc.nc

    with tc.tile_pool(name="persist", bufs=1) as pp, \
         tc.tile_pool(name="psum", bufs=2, space="PSUM") as psp:

        ident = pp.tile([P, P], f32, tag="ident")
        make_identity(nc, ident[:])
        # lowmask[p, k] = 1.0 where k < p, replicated BF times along free
        iot = pp.tile([P, P], i32, tag="iotpk")
        nc.gpsimd.iota(iot[:], pattern=[[1, P]], base=0, channel_multiplier=-1)
        lowm = pp.tile([P, BF, P], EQD, tag="lowmask")
        for j in range(BF):
            nc.vector.tensor_scalar(out=lowm[:, j, :], in0=iot[:], scalar1=0,
                                    scalar2=None, op0=AL.is_lt)
        ones_col = pp.tile([1, P], f32, tag="ones_col")
        nc.vector.memset(ones_col[:], 1.0)
        # trashc[p, j] = BF*NROWS + j*P + p  (unique in-call trash rows)
        tio = pp.tile([P, BF], i32, tag="tio")
        nc.gpsimd.iota(tio[:], pattern=[[P, BF]], base=BF * NROWS,
                       channel_multiplier=1)
        trashc = pp.tile([P, BF], f32, tag="trashc")
        nc.scalar.copy(trashc[:], tio[:])

        # ---- zero the grid tensors (HWDGE, overlaps with lift) ----
        with tc.tile_pool(name="zpool", bufs=1) as zp:
            ztile = zp.tile([P, 4096], GD, tag="zz")
            nc.vector.memset(ztile[:], 0.0)
            whole = GRID_ROWS * CG
            step = P * 4096
            for g in grids:
                gflat = g.rearrange("a c -> (a c)")
                starts = list(range(0, whole - step + 1, step))
                if starts[-1] + step < whole:
                    starts.append(whole - step)  # overlapped tail, re-zeroed
                for s in starts:
                    nc.sync.dma_start(
                        gflat[s:s + step].rearrange("(p m) -> p m", p=P),
                        ztile[:])

        # ---- pixel-constant tiles (shared by all cameras) ----
        wp_cm = tc.tile_pool(name="work", bufs=8)
        wp = wp_cm.__enter__()
        sp_cm = tc.tile_pool(name="setup", bufs=2)
        sp = sp_cm.__enter__()
        idl = pp.tile([P, FCAM], i32, tag="idl")
        nc.gpsimd.iota(idl[:], pattern=[[1, FCAM]], base=0, channel_multiplier=FCAM)
        idf = pp.tile([P, FCAM], f32, tag="idf")
        nc.scalar.copy(idf[:], idl[:])
        yf = pp.tile([P, FCAM], f32, tag="yf")
        xf = pp.tile([P, FCAM], f32, tag="xf")
        tmp = sp.tile([P, FCAM], f32, tag="t0")
        nc.scalar.activation(tmp[:], idf[:], ACT.Copy, bias=0.5 / W, scale=1.0 / W)
        _floor(nc, sp, yf[:], tmp[:], FCAM)
        nc.vector.scalar_tensor_tensor(out=xf[:], in0=yf[:], scalar=-float(W),
                                       in1=idf[:], op0=AL.mult, op1=AL.add)

        # region offset per chunk-column: regoff[p, f] = (f % BF) * NROWS
        fio = sp.tile([P, F], i32, tag="fio")
        nc.gpsimd.iota(fio[:], pattern=[[1, F]], base=0, channel_multiplier=0)
        fif = sp.tile([P, F], f32, tag="fif")
        nc.scalar.copy(fif[:], fio[:])
        fdiv = sp.tile([P, F], f32, tag="fdiv")
        nc.vector.tensor_scalar(out=fdiv[:], in0=fif[:], scalar1=1.0 / BF,
                                scalar2=None, op0=AL.mult)
        _floor(nc, sp, fdiv[:], fdiv[:], F)
        regoff = pp.tile([P, F], f32, tag="regoff")
        nc.vector.scalar_tensor_tensor(out=regoff[:], in0=fdiv[:],
                                       scalar=-float(BF), in1=fif[:],
                                       op0=AL.mult, op1=AL.add)
        nc.vector.tensor_scalar(out=regoff[:], in0=regoff[:],
                                scalar1=float(NROWS), scalar2=None, op0=AL.mult)

        # ---- per-camera geometry constants ----
        cam_consts = []
        for cam in range(NCAM):
            kc = pp.tile([3, 3], f32, tag=f"kc_{cam}")
            ec = pp.tile([4, 4], f32, tag=f"ec_{cam}")
            tmc = pp.tile([4, 4], f32, tag=f"tc_{cam}")
            nc.sync.dma_start(kc[:], kmat[cam])
            nc.sync.dma_start(ec[:], emat[cam])
            nc.sync.dma_start(tmc[:], tmat_t[cam])
            m4p = psp.tile([4, 4], f32, tag="smallp")
            nc.tensor.matmul(m4p[:], lhsT=tmc[:], rhs=ec[:],
                             start=True, stop=True)
            m4 = pp.tile([4, 4], f32, tag=f"m4_{cam}")
            nc.vector.tensor_copy(m4[:], m4p[:])
            m4tp = psp.tile([4, 4], f32, tag="smallp")
            nc.tensor.transpose(out=m4tp[:], in_=m4[:], identity=ident[:4, :4])
            m4t = pp.tile([4, 4], f32, tag=f"m4t_{cam}")
            nc.vector.tensor_copy(m4t[:], m4tp[:])
            n3p = psp.tile([3, 3], f32, tag="smallp")
            nc.tensor.matmul(n3p[:], lhsT=m4t[:3, :3], rhs=kc[:],
                             start=True, stop=True)
            n3 = pp.tile([3, 3], f32, tag=f"n3_{cam}")
            nc.vector.tensor_copy(n3[:], n3p[:])
            pk = pp.tile([1, 12], f32, tag=f"pk_{cam}")
            for i in range(3):
                nc.gpsimd.dma_start(pk[:, 3 * i:3 * i + 3], n3[i:i + 1, :])
                nc.gpsimd.dma_start(pk[:, 9 + i:10 + i], m4[i:i + 1, 3:4])
            bc = pp.tile([P, 12], f32, tag=f"bc_{cam}")
            nc.gpsimd.partition_broadcast(bc[:], pk[:])
            cam_consts.append(bc)

        # ---- load depths ----
        dtile = pp.tile([P, F], f32, tag="dtile")
        for cam in range(NCAM):
            cs = slice(cam * FCAM, (cam + 1) * FCAM)
            nc.sync.dma_start(
                dtile[:, cs],
                dep[cam * HW:(cam + 1) * HW].rearrange("(p f) -> p f", p=P))

        # ---- lift: voxel linear index per token ----
        linf = pp.tile([P, F], f32, tag="linf")
        nc.vector.memset(linf[:], INV)
        for cam in range(NCAM):
            cs = slice(cam * FCAM, (cam + 1) * FCAM)
            bc = cam_consts[cam]
            d = dtile[:, cs]
            gs = []
            for i in range(3):
                a = sp.tile([P, FCAM], f32, tag="a_i")
                nc.vector.tensor_scalar(out=a[:], in0=xf[:],
                                        scalar1=bc[:, 3 * i:3 * i + 1],
                                        scalar2=None, op0=AL.mult)
                nc.vector.scalar_tensor_tensor(out=a[:], in0=yf[:],
                                               scalar=bc[:, 3 * i + 1:3 * i + 2],
                                               in1=a[:], op0=AL.mult, op1=AL.add)
                nc.vector.tensor_scalar(out=a[:], in0=a[:],
                                        scalar1=bc[:, 3 * i + 2:3 * i + 3],
                                        scalar2=None, op0=AL.add)
                e = sp.tile([P, FCAM], f32, tag="e_i")
                nc.vector.tensor_tensor(out=e[:], in0=a[:], in1=d, op=AL.mult)
                nc.vector.tensor_scalar(out=e[:], in0=e[:],
                                        scalar1=bc[:, 9 + i:10 + i],
                                        scalar2=None, op0=AL.add)
                mid = (X / 2.0, Y / 2.0, Z / 2.0)[i]
                g = sp.tile([P, FCAM], f32, tag=f"g_{i}")
                nc.scalar.activation(g[:], e[:], ACT.Copy, bias=mid, scale=2.0)
                gs.append(g)
            gx, gy, gz = gs
            v = sp.tile([P, FCAM], f32, tag="v")
            nc.vector.tensor_scalar(out=v[:], in0=gx[:], scalar1=-1.0,
                                    scalar2=None, op0=AL.is_gt)
            nc.vector.scalar_tensor_tensor(out=v[:], in0=gx[:], scalar=float(X),
                                           in1=v[:], op0=AL.is_lt,
                                           op1=AL.logical_and)
            for gg, bound in ((gy, float(Y)), (gz, float(Z))):
                v2 = sp.tile([P, FCAM], f32, tag="v2")
                nc.vector.tensor_scalar(out=v2[:], in0=gg[:], scalar1=-1.0,
                                        scalar2=None, op0=AL.is_gt)
                nc.vector.scalar_tensor_tensor(out=v2[:], in0=gg[:], scalar=bound,
                                               in1=v2[:], op0=AL.is_lt,
                                               op1=AL.logical_and)
                nc.vector.tensor_tensor(out=v[:], in0=v[:], in1=v2[:],
                                        op=AL.logical_and)
            fx = sp.tile([P, FCAM], f32, tag="fx")
            fy = sp.tile([P, FCAM], f32, tag="fy")
            fz = sp.tile([P, FCAM], f32, tag="fz")
            _floor(nc, sp, fx[:], gx[:], FCAM)
            _floor(nc, sp, fy[:], gy[:], FCAM)
            _floor(nc, sp, fz[:], gz[:], FCAM)
            for ft in (fx, fy, fz):
                nc.vector.tensor_scalar(out=ft[:], in0=ft[:], scalar1=0.0,
                                        scalar2=255.0, op0=AL.max, op1=AL.min)
            lf = linf[:, cs]
            nc.vector.scalar_tensor_tensor(out=lf, in0=fz[:], scalar=float(X),
                                           in1=fx[:], op0=AL.mult, op1=AL.add)
            nc.vector.scalar_tensor_tensor(out=lf, in0=lf, scalar=float(Y),
                                           in1=fy[:], op0=AL.mult, op1=AL.add)
            # blend invalid -> INV :  lin = INV + v*(lin-INV)
            nc.vector.tensor_scalar(out=lf, in0=lf, scalar1=-INV,
                                    scalar2=None, op0=AL.add)
            nc.vector.tensor_tensor(out=lf, in0=lf, in1=v[:], op=AL.mult)
            nc.vector.tensor_scalar(out=lf, in0=lf, scalar1=INV,
                                    scalar2=None, op0=AL.add)
        # add per-chunk region offsets (invalid stays > BOUND)
        nc.vector.tensor_tensor(out=linf[:], in0=linf[:], in1=regoff[:],
                                op=AL.add)

        # ---- stage lin to DRAM: chunk-major [F,P] and partition-major [P,F]
        nc.sync.dma_start(lin_dram2[:, :], linf[:])
        NBLK = (F + P - 1) // P
        lin_t = pp.tile([P, NBLK, P], f32, tag="lin_t")
        for blk in range(NBLK):
            w = min(P, F - blk * P)
            ltp = psp.tile([P, P], f32, tag="tp")
            nc.tensor.transpose(out=ltp[:w, :], in_=linf[:, blk * P:blk * P + w],
                                identity=ident[:])
            nc.vector.tensor_copy(lin_t[:w, blk, :], ltp[:w, :])
            nc.sync.dma_start(lin_dram[blk * P:blk * P + w, :],
                              lin_t[:w, blk, :])

        sp_cm.__exit__(None, None, None)

        # ---- scatter: BF-chunk groups, one CCE-add call per group ----
        def group_body(gidx, alt):
            col4 = wp.tile([P, BF], f32, tag="col4")
            nc.sync.dma_start(col4[:], lin_dram2[:, bass.ts(gidx, BF)])
            payc = wp.tile([P, BF, CG], GD, tag="payc")
            nc.scalar.dma_start(
                payc[:], feat[bass.ts(gidx, BF * P), :]
                .rearrange("(k p) c -> p k c", p=P))
            # widerow[p, j*128+f] = id of token f of chunk j
            wrow = psp.tile([P, BF * P], f32, tag="wrow")
            if BF == 1:
                # transpose of the broadcast column == partition-broadcast row
                nc.tensor.transpose(out=wrow[:],
                                    in_=col4[:, 0:1].to_broadcast([P, P]),
                                    identity=ident[:])
            else:
                idrow = wp.tile([1, BF * P], f32, tag="idrow")
                nc.sync.dma_start(
                    idrow[:], lin_dram[bass.ts(gidx, BF), :]
                    .rearrange("a b -> (a b)")[None, :])
                nc.tensor.matmul(wrow[:], lhsT=ones_col[:], rhs=idrow[:],
                                 start=True, stop=True)
            eq = wp.tile([P, BF, P], EQD, tag="eq")
            for j in range(BF):
                nc.vector.tensor_tensor(
                    out=eq[:, j, :],
                    in0=col4[:, j:j + 1].to_broadcast([P, P]),
                    in1=wrow[:, j * P:(j + 1) * P], op=AL.is_equal)
            earl = wp.tile([P, BF, P], EQD, tag="earl")
            nc.vector.tensor_tensor(out=earl[:], in0=eq[:], in1=lowm[:],
                                    op=AL.mult)
            ecnt = wp.tile([P, BF], f32, tag="ecnt")
            nc.vector.tensor_reduce(out=ecnt[:], in_=earl[:],
                                    axis=mybir.AxisListType.X, op=AL.add)
            totp = psp.tile([P, BF * CG], f32, tag="totp")
            for j in range(BF):
                nc.tensor.matmul(totp[:, j * CG:(j + 1) * CG], lhsT=eq[:, j, :],
                                 rhs=payc[:, j, :], start=True, stop=True)
            tot = wp.tile([P, BF * CG], GD, tag="tot")
            nc.scalar.copy(tot[:], totp[:])
            # di = keep ? col : trash   (keep = first occurrence & valid)
            isval = wp.tile([P, BF], f32, tag="isval")
            nc.vector.tensor_scalar(out=isval[:], in0=col4[:], scalar1=INV,
                                    scalar2=None, op0=AL.is_lt)
            keep = wp.tile([P, BF], f32, tag="keep")
            nc.vector.scalar_tensor_tensor(out=keep[:], in0=ecnt[:], scalar=0.0,
                                           in1=isval[:], op0=AL.is_equal,
                                           op1=AL.logical_and)
            dif = wp.tile([P, BF], f32, tag="dif")
            nc.vector.tensor_tensor(out=dif[:], in0=col4[:], in1=trashc[:],
                                    op=AL.subtract)
            nc.vector.tensor_tensor(out=dif[:], in0=dif[:], in1=keep[:],
                                    op=AL.mult)
            nc.vector.tensor_tensor(out=dif[:], in0=dif[:], in1=trashc[:],
                                    op=AL.add)
            di = wp.tile([P, BF], i32, tag="di")
            nc.vector.tensor_copy(di[:], dif[:])
            nc.gpsimd.indirect_dma_start(
                out=grids[alt],
                out_offset=bass.IndirectOffsetOnAxis(ap=di[:], axis=0),
                in_=tot[:], in_offset=None,
                compute_op=AL.add)

        def unrollable_body(iv0, unroll):
            for i in range(unroll):
                group_body(iv0 + i, i % NT)

        tc.For_i_unrolled_general(start=0, end=G, step=1,
                                  unrollable_body=unrollable_body,
                                  max_unroll=GUNROLL)

        # ---- merge regions + z-max + store ----
        wp_cm.__exit__(None, None, None)
        SL = X * Y
        HF = SL // P // 2  # half-slice free length (256)
        with tc.tile_pool(name="merge", bufs=2) as mp, \
             tc.tile_pool(name="mload", bufs=6 if GD == bf16 else 3) as lp:
            for h in range(2):
                acc = mp.tile([P, HF, CG], GD, tag="acc")
                for z in range(Z):
                    base = z * SL + h * (SL // 2)
                    sz = mp.tile([P, HF, CG], GD, tag="sz")
                    szin = None
                    for r in range(BF):
                        for t in range(NT):
                            rows = slice(r * NROWS + base,
                                         r * NROWS + base + SL // 2)
                            ta = lp.tile([P, HF, CG], GD, tag="ta")
                            eng = nc.sync if (r * NT + t) % 2 == 0 else nc.scalar
                            eng.dma_start(
                                ta[:],
                                grids[t][rows, :]
                                .rearrange("(p f) c -> p f c", p=P))
                            if szin is None:
                                szin = ta
                            else:
                                nc.vector.tensor_tensor(out=sz[:], in0=szin[:],
                                                        in1=ta[:], op=AL.add)
                                szin = sz
                    if z == 0:
                        nc.vector.tensor_copy(acc[:], szin[:])
                    else:
                        nc.vector.tensor_tensor(out=acc[:], in0=acc[:],
                                                in1=szin[:], op=AL.max)
                accf = mp.tile([P, HF, CG], f32, tag="accf")
                nc.vector.tensor_copy(accf[:], acc[:])
                orows = slice(h * (SL // 2), (h + 1) * (SL // 2))
                nc.sync.dma_start(
                    out[orows, :].rearrange("(p f) c -> p f c", p=P), accf[:])


def kernel(feat_maps, depths, K, E, T):
    global _prog_cache
    feat_maps = np.asarray(feat_maps, np.float32)
    depths = np.asarray(depths, np.float32)
    K = np.asarray(K, np.float32)
    E = np.asarray(E, np.float32)
    T = np.asarray(T, np.float32)

    if _prog_cache is None:
        _prog_cache = _build_program()
    nc = _prog_cache

    in_maps = []
    for core in range(8):
        b, cg = core // 4, core % 4
        ch = slice(cg * CG, (cg + 1) * CG)
        frows = np.concatenate([
            np.ascontiguousarray(
                feat_maps[b * N + n, ch].transpose(1, 2, 0).reshape(HW, CG)
                .reshape(P, FCAM, CG).transpose(1, 0, 2).reshape(HW, CG))
            for n in range(N)], axis=0).astype(NPDT)
        cams = slice(b * N, (b + 1) * N)
        in_maps.append({
            "feat": frows,
            "dep": np.ascontiguousarray(depths[b].reshape(NTOK)),
            "kmat": np.ascontiguousarray(K[cams]),
            "emat": np.ascontiguousarray(E[cams]),
            "tmat_t": np.ascontiguousarray(T[cams].transpose(0, 2, 1)),
        })

    _tr = bool(os.environ.get("KBEV_TRACE"))
    res = bass_utils.run_bass_kernel_spmd(
        nc, in_maps, core_ids=list(range(8)), trace=_tr,
        trace_cores=(list(range(8)) if os.environ.get("KBEV_TRACE_ALL")
                     else [0]) if _tr else None)
    global last_result
    last_result = res
    outp = np.zeros((1, B * C, X, Y), np.float32)
    for core in range(8):
        b, cg = core // 4, core % 4
        o = np.asarray(res.results[core]["out"]).reshape(X, Y, CG)
        outp[0, b * C + cg * CG:b * C + (cg + 1) * CG] = \
            o.transpose(2, 0, 1)[:, ::-1, ::-1]
    return outp


# revision 22
# speedup vs baseline: 1.1414x; 1.0296x over previous
"""BEVLiftNet Trainium2 kernel (V3: direct CCE-add scatter chains).

Sharding: 8 cores = 2 batches x 4 channel-groups (16 ch each).
Each core lifts all 4 cameras of its batch (depths -> voxel ids), then
scatter-adds 16-channel f32 feature rows into NT round-robin DRAM grid
tensors via indirect DMA with CCE-add (compute_op=add), one call per
128-token chunk ([P,1] offsets -- the only offset shape the INDIRECT1D
ucode pairs correctly with payload; measured).  Concurrent CCE RMWs to
the same address lose updates (measured), so within-chunk duplicate
rows are pre-combined with an eq-matrix matmul (built against a PE
outer-product broadcast of the chunk ids, no per-chunk transpose) and
redirected to per-partition-unique trash rows past the voxel range.
Chunks round-robin over NT grid tensors so NT WAW chains pipeline and
gpsimd issues calls back-to-back (~1.7us fixed cost each).  A final
pass sums the grids, max-reduces over Z, and stores [X*Y, 16] f32; the
host reassembles the [1, B*C, X, Y] output.

Tuning knobs (env): KBEV_BF (chunks/call; >1 requires multi-column
offset support the current ucode lacks), KBEV_NT (grid tensors),
KBEV_GD (grid dtype), KBEV_UNROLL (chunks per hw-loop iteration).
"""

import os
import sys

sys.path.insert(0, "/opt/trn_rl_repo")

import ml_dtypes
import numpy as np

import concourse.bacc as bacc
import concourse.bass as bass
import concourse.mybir as mybir
import concourse.tile as tile
from concourse import bass_utils
from concourse.masks import make_identity

B, N, C, H, W = 2, 4, 64, 128, 352
X, Y, Z = 256, 256, 8
CG = 16           # channels per core
NCAM = 4          # cameras per core (one batch)
HW = H * W        # 45056 pixels per camera
NTOK = NCAM * HW  # 180224 tokens per core
P = 128
FCAM = HW // P    # 352 chunk-columns per camera
F = NCAM * FCAM   # 1408 chunk-columns total
NROWS = Z * X * Y          # 524288 voxel rows per region
f32 = mybir.dt.float32
bf16 = mybir.dt.bfloat16
i32 = mybir.dt.int32
AL = mybir.AluOpType
ACT = mybir.ActivationFunctionType

BF = int(os.environ.get("KBEV_BF", "1"))        # chunks (regions) per call
NT = int(os.environ.get("KBEV_NT", "4"))        # alternating grid tensors
GD = bf16 if os.environ.get("KBEV_GD", "f32") == "bf16" else f32
GUNROLL = int(os.environ.get("KBEV_UNROLL", "128"))  # groups per loop iter
G = F // BF                                     # scatter groups (calls)
GRID_ROWS = BF * NROWS + BF * P   # + per-(chunk,partition) trash rows
INV = float(BF * NROWS)   # invalid-token id marker (>= all valid ids)
NPDT = ml_dtypes.bfloat16 if GD == bf16 else np.float32
EQD = GD

_prog_cache = None


def _build_program():
    nc = bacc.Bacc("TRN2", target_bir_lowering=False, debug=False)
    feat = nc.dram_tensor("feat", [NTOK, CG], GD, kind="ExternalInput")
    dep = nc.dram_tensor("dep", [NTOK], f32, kind="ExternalInput")
    kmat = nc.dram_tensor("kmat", [NCAM, 3, 3], f32, kind="ExternalInput")
    emat = nc.dram_tensor("emat", [NCAM, 4, 4], f32, kind="ExternalInput")
    tmat_t = nc.dram_tensor("tmat_t", [NCAM, 4, 4], f32, kind="ExternalInput")
    out = nc.dram_tensor("out", [X * Y, CG], f32, kind="ExternalOutput")
    grids = [nc.dram_tensor(f"grid{t}", [GRID_ROWS, CG], GD, kind="Internal")
             for t in range(NT)]
    lin_dram = nc.dram_tensor("lin_dram", [F, P], f32, kind="Internal")
    lin_dram2 = nc.dram_tensor("lin_dram2", [P, F], f32, kind="Internal")
    with tile.TileContext(nc) as tc:
        _emit(tc, feat.ap(), dep.ap(), kmat.ap(), emat.ap(), tmat_t.ap(),
              out.ap(), [g.ap() for g in grids], lin_dram.ap(), lin_dram2.ap())
    nc.compile()
    return nc


def _floor(nc, wp, out_t, in_t, n):
    """out = floor(in), robust to any f32->i32 rounding mode."""
    ii = wp.tile([P, n], i32, tag="fl_i")
    nc.vector.tensor_copy(ii[:], in_t)
    ff = wp.tile([P, n], f32, tag="fl_f")
    nc.vector.tensor_copy(ff[:], ii[:])
    gt = wp.tile([P, n], f32, tag="fl_g")
    nc.vector.tensor_tensor(out=gt[:], in0=ff[:], in1=in_t, op=AL.is_gt)
    nc.vector.tensor_tensor(out=out_t, in0=ff[:], in1=gt[:], op=AL.subtract)


def _emit(tc, feat, dep, kmat, emat, tmat_t, out, grids, lin_dram, lin_dram2):
    nc = tc.nc

    with tc.tile_pool(name="persist", bufs=1) as pp, \
         tc.tile_pool(name="psum", bufs=2, space="PSUM") as psp:

        ident = pp.tile([P, P], f32, tag="ident")
        make_identity(nc, ident[:])
        # lowmask[p, k] = 1.0 where k < p, replicated BF times along free
        iot = pp.tile([P, P], i32, tag="iotpk")
        nc.gpsimd.iota(iot[:], pattern=[[1, P]], base=0, channel_multiplier=-1)
        lowm = pp.tile([P, BF, P], EQD, tag="lowmask")
        for j in range(BF):
            nc.vector.tensor_scalar(out=lowm[:, j, :], in0=iot[:], scalar1=0,
                                    scalar2=None, op0=AL.is_lt)
        ones_col = pp.tile([1, P], f32, tag="ones_col")
        nc.vector.memset(ones_col[:], 1.0)
        # trashc[p, j] = BF*NROWS + j*P + p  (unique in-call trash rows)
        tio = pp.tile([P, BF], i32, tag="tio")
        nc.gpsimd.iota(tio[:], pattern=[[P, BF]], base=BF * NROWS,
                       channel_multiplier=1)
        trashc = pp.tile([P, BF], f32, tag="trashc")
        nc.scalar.copy(trashc[:], tio[:])

        # ---- zero the grid tensors (HWDGE, overlaps with lift) ----
        with tc.tile_pool(name="zpool", bufs=1) as zp:
            ztile = zp.tile([P, 4096], GD, tag="zz")
            nc.vector.memset(ztile[:], 0.0)
            whole = GRID_ROWS * CG
            step = P * 4096
            for g in grids:
                gflat = g.rearrange("a c -> (a c)")
                starts = list(range(0, whole - step + 1, step))
                if starts[-1] + step < whole:
                    starts.append(whole - step)  # overlapped tail, re-zeroed
                for s in starts:
                    nc.sync.dma_start(
                        gflat[s:s + step].rearrange("(p m) -> p m", p=P),
                        ztile[:])

        # ---- pixel-constant tiles (shared by all cameras) ----
        wp_cm = tc.tile_pool(name="work", bufs=8)
        wp = wp_cm.__enter__()
        sp_cm = tc.tile_pool(name="setup", bufs=2)
        sp = sp_cm.__enter__()
        idl = pp.tile([P, FCAM], i32, tag="idl")
        nc.gpsimd.iota(idl[:], pattern=[[1, FCAM]], base=0, channel_multiplier=FCAM)
        idf = pp.tile([P, FCAM], f32, tag="idf")
        nc.scalar.copy(idf[:], idl[:])
        yf = pp.tile([P, FCAM], f32, tag="yf")
        xf = pp.tile([P, FCAM], f32, tag="xf")
        tmp = sp.tile([P, FCAM], f32, tag="t0")
        nc.scalar.activation(tmp[:], idf[:], ACT.Copy, bias=0.5 / W, scale=1.0 / W)
        _floor(nc, sp, yf[:], tmp[:], FCAM)
        nc.vector.scalar_tensor_tensor(out=xf[:], in0=yf[:], scalar=-float(W),
                                       in1=idf[:], op0=AL.mult, op1=AL.add)

        # region offset per chunk-column: regoff[p, f] = (f % BF) * NROWS
        fio = sp.tile([P, F], i32, tag="fio")
        nc.gpsimd.iota(fio[:], pattern=[[1, F]], base=0, channel_multiplier=0)
        fif = sp.tile([P, F], f32, tag="fif")
        nc.scalar.copy(fif[:], fio[:])
        fdiv = sp.tile([P, F], f32, tag="fdiv")
        nc.vector.tensor_scalar(out=fdiv[:], in0=fif[:], scalar1=1.0 / BF,
                                scalar2=None, op0=AL.mult)
        _floor(nc, sp, fdiv[:], fdiv[:], F)
        regoff = pp.tile([P, F], f32, tag="regoff")
        nc.vector.scalar_tensor_tensor(out=regoff[:], in0=fdiv[:],
                                       scalar=-float(BF), in1=fif[:],
                                       op0=AL.mult, op1=AL.add)
        nc.vector.tensor_scalar(out=regoff[:], in0=regoff[:],
                                scalar1=float(NROWS), scalar2=None, op0=AL.mult)

        # ---- per-camera geometry constants ----
        cam_consts = []
        for cam in range(NCAM):
            kc = pp.tile([3, 3], f32, tag=f"kc_{cam}")
            ec = pp.tile([4, 4], f32, tag=f"ec_{cam}")
            tmc = pp.tile([4, 4], f32, tag=f"tc_{cam}")
            nc.sync.dma_start(kc[:], kmat[cam])
            nc.sync.dma_start(ec[:], emat[cam])
            nc.sync.dma_start(tmc[:], tmat_t[cam])
            m4p = psp.tile([4, 4], f32, tag="smallp")
            nc.tensor.matmul(m4p[:], lhsT=tmc[:], rhs=ec[:],
                             start=True, stop=True)
            m4 = pp.tile([4, 4], f32, tag=f"m4_{cam}")
            nc.vector.tensor_copy(m4[:], m4p[:])
            m4tp = psp.tile([4, 4], f32, tag="smallp")
            nc.tensor.transpose(out=m4tp[:], in_=m4[:], identity=ident[:4, :4])
            m4t = pp.tile([4, 4], f32, tag=f"m4t_{cam}")
            nc.vector.tensor_copy(m4t[:], m4tp[:])
            n3p = psp.tile([3, 3], f32, tag="smallp")
            nc.tensor.matmul(n3p[:], lhsT=m4t[:3, :3], rhs=kc[:],
                             start=True, stop=True)
            n3 = pp.tile([3, 3], f32, tag=f"n3_{cam}")
            nc.vector.tensor_copy(n3[:], n3p[:])
            pk = pp.tile([1, 12], f32, tag=f"pk_{cam}")
            for i in range(3):
                nc.gpsimd.dma_start(pk[:, 3 * i:3 * i + 3], n3[i:i + 1, :])
                nc.gpsimd.dma_start(pk[:, 9 + i:10 + i], m4[i:i + 1, 3:4])
            bc = pp.tile([P, 12], f32, tag=f"bc_{cam}")
            nc.gpsimd.partition_broadcast(bc[:], pk[:])
            cam_consts.append(bc)

        # ---- load depths ----
        dtile = pp.tile([P, F], f32, tag="dtile")
        for cam in range(NCAM):
            cs = slice(cam * FCAM, (cam + 1) * FCAM)
            nc.sync.dma_start(
                dtile[:, cs],
                dep[cam * HW:(cam + 1) * HW].rearrange("(p f) -> p f", p=P))

        # ---- lift: voxel linear index per token ----
        linf = pp.tile([P, F], f32, tag="linf")
        nc.vector.memset(linf[:], INV)
        for cam in range(NCAM):
            cs = slice(cam * FCAM, (cam + 1) * FCAM)
            bc = cam_consts[cam]
            d = dtile[:, cs]
            gs = []
            for i in range(3):
                a = sp.tile([P, FCAM], f32, tag="a_i")
                nc.vector.tensor_scalar(out=a[:], in0=xf[:],
                                        scalar1=bc[:, 3 * i:3 * i + 1],
                                        scalar2=None, op0=AL.mult)
                nc.vector.scalar_tensor_tensor(out=a[:], in0=yf[:],
                                               scalar=bc[:, 3 * i + 1:3 * i + 2],
                                               in1=a[:], op0=AL.mult, op1=AL.add)
                nc.vector.tensor_scalar(out=a[:], in0=a[:],
                                        scalar1=bc[:, 3 * i + 2:3 * i + 3],
                                        scalar2=None, op0=AL.add)
                e = sp.tile([P, FCAM], f32, tag="e_i")
                nc.vector.tensor_tensor(out=e[:], in0=a[:], in1=d, op=AL.mult)
                nc.vector.tensor_scalar(out=e[:], in0=e[:],
                                        scalar1=bc[:, 9 + i:10 + i],
                                        scalar2=None, op0=AL.add)
                mid = (X / 2.0, Y / 2.0, Z / 2.0)[i]
                g = sp.tile([P, FCAM], f32, tag=f"g_{i}")
                nc.scalar.activation(g[:], e[:], ACT.Copy, bias=mid, scale=2.0)
                gs.append(g)
            gx, gy, gz = gs
            v = sp.tile([P, FCAM], f32, tag="v")
            nc.vector.tensor_scalar(out=v[:], in0=gx[:], scalar1=-1.0,
                                    scalar2=None, op0=AL.is_gt)
            nc.vector.scalar_tensor_tensor(out=v[:], in0=gx[:], scalar=float(X),
                                           in1=v[:], op0=AL.is_lt,
                                           op1=AL.logical_and)
            for gg, bound in ((gy, float(Y)), (gz, float(Z))):
                v2 = sp.tile([P, FCAM], f32, tag="v2")
                nc.vector.tensor_scalar(out=v2[:], in0=gg[:], scalar1=-1.0,
                                        scalar2=None, op0=AL.is_gt)
                nc.vector.scalar_tensor_tensor(out=v2[:], in0=gg[:], scalar=bound,
                                               in1=v2[:], op0=AL.is_lt,
                                               op1=AL.logical_and)
                nc.vector.tensor_tensor(out=v[:], in0=v[:], in1=v2[:],
                                        op=AL.logical_and)
            fx = sp.tile([P, FCAM], f32, tag="fx")
            fy = sp.tile([P, FCAM], f32, tag="fy")
            fz = sp.tile([P, FCAM], f32, tag="fz")
            _floor(nc, sp, fx[:], gx[:], FCAM)
            _floor(nc, sp, fy[:], gy[:], FCAM)
            _floor(nc, sp, fz[:], gz[:], FCAM)
            for ft in (fx, fy, fz):
                nc.vector.tensor_scalar(out=ft[:], in0=ft[:], scalar1=0.0,
                                        scalar2=255.0, op0=AL.max, op1=AL.min)
            lf = linf[:, cs]
            nc.vector.scalar_tensor_tensor(out=lf, in0=fz[:], scalar=float(X),
                                           in1=fx[:], op0=AL.mult, op1=AL.add)
            nc.vector.scalar_tensor_tensor(out=lf, in0=lf, scalar=float(Y),
                                           in1=fy[:], op0=AL.mult, op1=AL.add)
            # blend invalid -> INV :  lin = INV + v*(lin-INV)
            nc.vector.tensor_scalar(out=lf, in0=lf, scalar1=-INV,
                                    scalar2=None, op0=AL.add)
            nc.vector.tensor_tensor(out=lf, in0=lf, in1=v[:], op=AL.mult)
            nc.vector.tensor_scalar(out=lf, in0=lf, scalar1=INV,
                                    scalar2=None, op0=AL.add)
        # add per-chunk region offsets (invalid stays > BOUND)
        nc.vector.tensor_tensor(out=linf[:], in0=linf[:], in1=regoff[:],
                                op=AL.add)

        # ---- stage lin to DRAM: chunk-major [F,P] and partition-major [P,F]
        nc.sync.dma_start(lin_dram2[:, :], linf[:])
        NBLK = (F + P - 1) // P
        lin_t = pp.tile([P, NBLK, P], f32, tag="lin_t")
        for blk in range(NBLK):
            w = min(P, F - blk * P)
            ltp = psp.tile([P, P], f32, tag="tp")
            nc.tensor.transpose(out=ltp[:w, :], in_=linf[:, blk * P:blk * P + w],
                                identity=ident[:])
            nc.vector.tensor_copy(lin_t[:w, blk, :], ltp[:w, :])
            nc.sync.dma_start(lin_dram[blk * P:blk * P + w, :],
                              lin_t[:w, blk, :])

        sp_cm.__exit__(None, None, None)

        # ---- scatter: BF-chunk groups, one CCE-add call per group ----
        def group_body(gidx, alt):
            col4 = wp.tile([P, BF], f32, tag="col4")
            nc.sync.dma_start(col4[:], lin_dram2[:, bass.ts(gidx, BF)])
            payc = wp.tile([P, BF, CG], GD, tag="payc")
            nc.scalar.dma_start(
                payc[:], feat[bass.ts(gidx, BF * P), :]
                .rearrange("(k p) c -> p k c", p=P))
            # widerow[p, j*128+f] = id of token f of chunk j
            wrow = psp.tile([P, BF * P], f32, tag="wrow")
            if BF == 1:
                # transpose of the broadcast column == partition-broadcast row
                nc.tensor.transpose(out=wrow[:],
                                    in_=col4[:, 0:1].to_broadcast([P, P]),
                                    identity=ident[:])
            else:
                idrow = wp.tile([1, BF * P], f32, tag="idrow")
                nc.sync.dma_start(
                    idrow[:], lin_dram[bass.ts(gidx, BF), :]
                    .rearrange("a b -> (a b)")[None, :])
                nc.tensor.matmul(wrow[:], lhsT=ones_col[:], rhs=idrow[:],
                                 start=True, stop=True)
            eq = wp.tile([P, BF, P], EQD, tag="eq")
            for j in range(BF):
                nc.vector.tensor_tensor(
                    out=eq[:, j, :],
                    in0=col4[:, j:j + 1].to_broadcast([P, P]),
                    in1=wrow[:, j * P:(j + 1) * P], op=AL.is_equal)
            earl = wp.tile([P, BF, P], EQD, tag="earl")
            nc.vector.tensor_tensor(out=earl[:], in0=eq[:], in1=lowm[:],
                                    op=AL.mult)
            ecnt = wp.tile([P, BF], f32, tag="ecnt")
            nc.vector.tensor_reduce(out=ecnt[:], in_=earl[:],
                                    axis=mybir.AxisListType.X, op=AL.add)
            totp = psp.tile([P, BF * CG], f32, tag="totp")
            for j in range(BF):
                nc.tensor.matmul(totp[:, j * CG:(j + 1) * CG], lhsT=eq[:, j, :],
                                 rhs=payc[:, j, :], start=True, stop=True)
            tot = wp.tile([P, BF * CG], GD, tag="tot")
            nc.scalar.copy(tot[:], totp[:])
            # di = keep ? col : trash   (keep = first occurrence & valid)
            isval = wp.tile([P, BF], f32, tag="isval")
            nc.vector.tensor_scalar(out=isval[:], in0=col4[:], scalar1=INV,
                                    scalar2=None, op0=AL.is_lt)
            keep = wp.tile([P, BF], f32, tag="keep")
            nc.vector.scalar_tensor_tensor(out=keep[:], in0=ecnt[:], scalar=0.0,
                                           in1=isval[:], op0=AL.is_equal,
                                           op1=AL.logical_and)
            dif = wp.tile([P, BF], f32, tag="dif")
            nc.vector.tensor_tensor(out=dif[:], in0=col4[:], in1=trashc[:],
                                    op=AL.subtract)
            nc.vector.tensor_tensor(out=dif[:], in0=dif[:], in1=keep[:],
                                    op=AL.mult)
            nc.vector.tensor_tensor(out=dif[:], in0=dif[:], in1=trashc[:],
                                    op=AL.add)
            di = wp.tile([P, BF], i32, tag="di")
            nc.vector.tensor_copy(di[:], dif[:])
            nc.gpsimd.indirect_dma_start(
                out=grids[alt],
                out_offset=bass.IndirectOffsetOnAxis(ap=di[:], axis=0),
                in_=tot[:], in_offset=None,
                compute_op=AL.add)

        def unrollable_body(iv0, unroll):
            for i in range(unroll):
                group_body(iv0 + i, i % NT)

        tc.For_i_unrolled_general(start=0, end=G, step=1,
                                  unrollable_body=unrollable_body,
                                  max_unroll=GUNROLL)

        # ---- merge regions + z-max + store ----
        wp_cm.__exit__(None, None, None)
        SL = X * Y
        HF = SL // P // 2  # half-slice free length (256)
        with tc.tile_pool(name="merge", bufs=2) as mp, \
             tc.tile_pool(name="mload", bufs=8 if GD == bf16 else 5) as lp:
            for h in range(2):
                acc = mp.tile([P, HF, CG], GD, tag="acc")
                for z in range(Z):
                    base = z * SL + h * (SL // 2)
                    sz = mp.tile([P, HF, CG], GD, tag="sz")
                    szin = None
                    for r in range(BF):
                        for t in range(NT):
                            rows = slice(r * NROWS + base,
                                         r * NROWS + base + SL // 2)
                            ta = lp.tile([P, HF, CG], GD, tag="ta")
                            eng = nc.sync if (r * NT + t) % 2 == 0 else nc.scalar
                            eng.dma_start(
                                ta[:],
                                grids[t][rows, :]
                                .rearrange("(p f) c -> p f c", p=P))
                            if szin is None:
                                szin = ta
                            else:
                                nc.vector.tensor_tensor(out=sz[:], in0=szin[:],
                                                        in1=ta[:], op=AL.add)
                                szin = sz
                    if z == 0:
                        nc.vector.tensor_copy(acc[:], szin[:])
                    else:
                        nc.vector.tensor_tensor(out=acc[:], in0=acc[:],
                                                in1=szin[:], op=AL.max)
                accf = mp.tile([P, HF, CG], f32, tag="accf")
                nc.vector.tensor_copy(accf[:], acc[:])
                orows = slice(h * (SL // 2), (h + 1) * (SL // 2))
                nc.sync.dma_start(
                    out[orows, :].rearrange("(p f) c -> p f c", p=P), accf[:])


def kernel(feat_maps, depths, K, E, T):
    global _prog_cache
    feat_maps = np.asarray(feat_maps, np.float32)
    depths = np.asarray(depths, np.float32)
    K = np.asarray(K, np.float32)
    E = np.asarray(E, np.float32)
    T = np.asarray(T, np.float32)

    if _prog_cache is None:
        _prog_cache = _build_program()
    nc = _prog_cache

    in_maps = []
    for core in range(8):
        b, cg = core // 4, core % 4
        ch = slice(cg * CG, (cg + 1) * CG)
        frows = np.concatenate([
            np.ascontiguousarray(
                feat_maps[b * N + n, ch].transpose(1, 2, 0).reshape(HW, CG)
                .reshape(P, FCAM, CG).transpose(1, 0, 2).reshape(HW, CG))
            for n in range(N)], axis=0).astype(NPDT)
        cams = slice(b * N, (b + 1) * N)
        in_maps.append({
            "feat": frows,
            "dep": np.ascontiguousarray(depths[b].reshape(NTOK)),
            "kmat": np.ascontiguousarray(K[cams]),
            "emat": np.ascontiguousarray(E[cams]),
            "tmat_t": np.ascontiguousarray(T[cams].transpose(0, 2, 1)),
        })

    _tr = bool(os.environ.get("KBEV_TRACE"))
    res = bass_utils.run_bass_kernel_spmd(
        nc, in_maps, core_ids=list(range(8)), trace=_tr,
        trace_cores=(list(range(8)) if os.environ.get("KBEV_TRACE_ALL")
                     else [0]) if _tr else None)
    global last_result
    last_result = res
    outp = np.zeros((1, B * C, X, Y), np.float32)
    for core in range(8):
        b, cg = core // 4, core % 4
        o = np.asarray(res.results[core]["out"]).reshape(X, Y, CG)
        outp[0, b * C + cg * CG:b * C + (cg + 1) * CG] = \
            o.transpose(2, 0, 1)[:, ::-1, ::-1]
    return outp
